# revision 1
# baseline (speedup 1.0000x reference)
"""GAT (2-layer, PyG-style) Trainium2 Bass kernel, 8-core SPMD.

Strategy (dst-sharded graph parallel):
  - Nodes sharded by dst range (6250/core). Edges (incl. self-loops) bucketed
    per core by dst, sorted, grouped into 8-slot sub-chunks (one dst each),
    128 sub-chunks = 1 super-chunk (SC). Two streams per core: src<HALF (A)
    and src>=HALF (B) so gather indices fit int16.
  - Node pass computes, per node, h = x@W and the attention exponentials
    es=exp(s), es02=exp(0.2 s), ed=exp(d), ed02=exp(0.2 d) via a fused matmul
    x @ [W | Ws | 0.2Ws | Wd | 0.2Wd]; rows packed into 256B bf16 gather
    tables.  Key identity (exact for leaky_relu slope 0.2):
        exp(leaky_relu(s+d)) = max(es*ed, min(es02*ed02, 1))
    which factorizes src/dst terms and avoids any per-edge transcendentals.
  - Edge pass per SC: dma_gather rows by src (h,es,es02), dma_gather one row
    per sub-chunk by dst (ed,ed02), compute ex and ex-weighted h rows, then a
    membership matmul (lhsT = one-hot of dst-slot built by is_equal against an
    iota tile) accumulates per-dst sums + denominators in PSUM; result rows are
    dma_scatter_add-ed (CCE add) into a per-stream DRAM accumulator.
  - Between layers: normalize, bias, ELU, second fused matmul, AllGather of
    the per-core table slices, then the same edge pass for layer 2; tail
    applies softmax-denominator, bias and log_softmax.
"""

import math
import os
import sys

import numpy as np

sys.path.insert(0, "/opt/trn_rl_repo")

import ml_dtypes

BF16 = ml_dtypes.bfloat16

# ---------------------------------------------------------------- problem cfg
N_NODES = 50000
N_EDGES = 1600000
IN_CH = 128
C1 = 64  # heads1*dim1
H1 = 8
D1H = 8
C2 = 32
H2 = 1
NEG = 0.2
EPS = 1e-16
NCORES = 8
HALF = 25000  # src-half split so gather idx fits int16 (+1 zero row)
S = 8  # slots per sub-chunk
SCP = 128  # sub-chunks per super-chunk


class Cfg:
    def __init__(self, n_nodes=N_NODES, in_ch=IN_CH, c1=C1, h1=H1, c2=C2,
                 ncores=NCORES, half=HALF):
        self.n_nodes = n_nodes
        self.in_ch = in_ch
        self.c1 = c1
        self.h1 = h1
        self.d1 = c1 // h1
        self.c2 = c2
        self.ncores = ncores
        self.half = half
        self.rows_core = n_nodes // ncores
        self.acc_rows = ((self.rows_core + 127) // 128) * 128 + 256  # + park
        self.park = self.acc_rows - 192
        self.na = half + 1          # table-a rows (zero row at half)
        self.nb = n_nodes - half + 1


# ------------------------------------------------------------- host edge plan
def _wrap_idx(idx, reps=128):
    """[n] -> wrapped [16, n/16] layout (pos i at [i%16, i//16]), replicated."""
    n = idx.shape[0]
    w = idx.reshape(n // 16, 16).T.copy()  # [16, n/16]
    return np.tile(w, (reps // 16, 1))


def build_plan(src, dst, cfg: Cfg):
    """Per-core, per-stream super-chunk plan. Returns meta [cores, SCN, 128, 81]
    int16 and (scA, scB)."""
    rows = cfg.rows_core
    half = cfg.half
    core_of = dst // rows
    per_core = []
    for c in range(cfg.ncores):
        m = core_of == c
        s_c = src[m].astype(np.int64)
        d_c = dst[m].astype(np.int64) - c * rows
        stream = (s_c >= half).astype(np.int64)
        order = np.lexsort((s_c, stream, d_c))
        s_c, d_c, stream = s_c[order], d_c[order], stream[order]
        per_core.append((s_c, d_c, stream))

    # sub-chunk lists per (core, stream)
    def subchunks(s_c, d_c, st_c, which):
        m = st_c == which
        s, d = s_c[m], d_c[m]
        if s.shape[0] == 0:
            return (np.zeros((0, S), np.int64), np.zeros((0,), np.int64))
        # group by dst (sorted); ranks within group
        chg = np.r_[True, d[1:] != d[:-1]]
        gid = np.cumsum(chg) - 1
        gstart = np.flatnonzero(chg)
        rank = np.arange(d.shape[0]) - gstart[gid]
        sub_l = rank // S          # sub-chunk index within group
        slot = rank % S
        gsub = np.zeros(gid.max() + 1, np.int64)
        np.maximum.at(gsub, gid, sub_l + 1)          # sub-chunks per group
        gsub_off = np.r_[0, np.cumsum(gsub)]
        subid = gsub_off[gid] + sub_l
        nsub = int(gsub_off[-1])
        src_slots = np.zeros((nsub, S), np.int64)    # pad -> zero row
        zr = half if which == 0 else (cfg.n_nodes - half)
        src_slots[:] = zr
        rel = s - (0 if which == 0 else half)
        src_slots[subid, slot] = rel
        sub_dst = np.repeat(d[gstart], gsub)         # dst_rel per sub-chunk
        return src_slots, sub_dst

    plans = []  # per core: list of SC dicts per stream
    maxsc = [0, 0]
    for c in range(cfg.ncores):
        s_c, d_c, st_c = per_core[c]
        streams = []
        for which in (0, 1):
            src_slots, sub_dst = subchunks(s_c, d_c, st_c, which)
            # pack whole dst-groups (consecutive equal sub_dst) into SCs <=128
            scs = []
            n = sub_dst.shape[0]
            i = 0
            cur = []  # list of (start, count) groups
            cur_n = 0
            while i < n:
                j = i
                while j < n and sub_dst[j] == sub_dst[i]:
                    j += 1
                g = j - i
                assert g <= SCP, "dst run too large for one super-chunk"
                if cur_n + g > SCP:
                    scs.append((cur, cur_n))
                    cur, cur_n = [], 0
                cur.append((i, g))
                cur_n += g
                i = j
            if cur_n:
                scs.append((cur, cur_n))
            streams.append((src_slots, sub_dst, scs))
            maxsc[which] = max(maxsc[which], len(scs))
        plans.append(streams)

    scA = ((maxsc[0] + 3) // 4) * 4
    scB = ((maxsc[1] + 3) // 4) * 4
    maxsc = [scA, scB]
    scn = scA + scB
    meta = np.zeros((cfg.ncores, scn, 128, 81), np.int16)
    dsl = np.full((cfg.ncores, scn, 128), -1.0, np.float32)
    for c in range(cfg.ncores):
        for which in (0, 1):
            src_slots, sub_dst, scs = plans[c][which]
            zr = cfg.half if which == 0 else (cfg.n_nodes - cfg.half)
            base = 0 if which == 0 else scA
            nsc = scA if which == 0 else scB
            for k in range(nsc):
                g_idx = np.full((128, S), zr, np.int64)
                d_idx = np.full((128,), cfg.park, np.int64)
                dstslot = np.full((128,), -1.0, np.float64)
                scat = np.full((128,), 0, np.int64)
                scat[:] = cfg.park + np.arange(128) % 64
                if k < len(scs):
                    groups, _n = scs[k]
                    p = 0
                    sid = 0
                    for (gs, gc) in groups:
                        g_idx[p:p + gc] = src_slots[gs:gs + gc]
                        d_idx[p:p + gc] = sub_dst[gs]
                        dstslot[p:p + gc] = sid
                        scat[sid] = sub_dst[gs]
                        p += gc
                        sid += 1
                mrow = meta[c, base + k]
                # gather idx list: position i = slot*128 + p
                flat = g_idx.T.reshape(-1)  # [1024] pos-ordered
                mrow[:, 0:64] = _wrap_idx(flat.astype(np.int16))
                mrow[:, 64:72] = _wrap_idx(d_idx.astype(np.int16))
                mrow[:, 72:80] = _wrap_idx(scat.astype(np.int16))
                mrow[:, 80] = 0
                dsl[c, base + k] = dstslot.astype(np.float32)
    return meta, dsl, scA, scB


# --------------------------------------------------------------- bass program
def build_program(cfg: Cfg, scA, scB, with_cc=True):
    from concourse import bacc, bass, library_config, mybir, tile
    from concourse.masks import make_identity

    f32 = mybir.dt.float32
    bf16 = mybir.dt.bfloat16
    i16 = mybir.dt.int16
    Alu = mybir.AluOpType
    Act = mybir.ActivationFunctionType

    scn = scA + scB
    nc = bacc.Bacc(None, target_bir_lowering=False, debug=False)

    # ---- I/O
    x_in = nc.dram_tensor("x_in", [cfg.n_nodes, cfg.in_ch], f32, kind="ExternalInput")
    x_own = nc.dram_tensor("x_own", [cfg.rows_core, cfg.in_ch], f32, kind="ExternalInput")
    wc1 = nc.dram_tensor("wc1", [cfg.in_ch, 96], f32, kind="ExternalInput")
    wc2 = nc.dram_tensor("wc2", [cfg.c1, 36], f32, kind="ExternalInput")
    b1r = nc.dram_tensor("b1r", [128, cfg.c1], f32, kind="ExternalInput")
    b2r = nc.dram_tensor("b2r", [128, cfg.c2], f32, kind="ExternalInput")
    iota_in = nc.dram_tensor("iota_in", [128, 128], f32, kind="ExternalInput")
    scg = scn // 4
    meta_in = nc.dram_tensor("meta_in", [scg, 128, 328], i16, kind="ExternalInput")
    out_ext = nc.dram_tensor("out_ext", [cfg.rows_core, cfg.c2], f32, kind="ExternalOutput")

    # ---- internal DRAM
    t1a = nc.dram_tensor("t1a", [cfg.na, 128], bf16)
    t1b = nc.dram_tensor("t1b", [cfg.nb, 128], bf16)
    t2a = nc.dram_tensor("t2a", [cfg.na, 128], bf16)
    t2b = nc.dram_tensor("t2b", [cfg.nb, 128], bf16)
    d1t = nc.dram_tensor("d1t", [cfg.acc_rows, 128], bf16)
    d2t = nc.dram_tensor("d2t", [cfg.acc_rows, 128], bf16)
    accs = {n: nc.dram_tensor(n, [cfg.acc_rows, 128], f32)
            for n in ("a1A", "a1B", "a2A", "a2B")}
    agin = nc.dram_tensor("agin", [cfg.rows_core, 34], bf16)
    agout = nc.dram_tensor("agout", [cfg.n_nodes, 34], bf16, addr_space="Shared")

    ntile = (cfg.n_nodes + 127) // 128
    otile = (cfg.rows_core + 127) // 128

    with tile.TileContext(nc) as tc:
        nc.gpsimd.load_library(library_config.mlp)
        with tc.tile_pool(name="const", bufs=1) as cpool:
            ident = cpool.tile([128, 128], f32)
            make_identity(nc, ident[:])
            wc1_s = cpool.tile([cfg.in_ch, 96], f32)
            nc.sync.dma_start(wc1_s[:], wc1[:, :])
            wc2_s = cpool.tile([cfg.c1, 36], f32)
            nc.sync.dma_start(wc2_s[:], wc2[:, :])
            b1_s = cpool.tile([128, cfg.c1], f32)
            nc.sync.dma_start(b1_s[:], b1r[:, :])
            b2_s = cpool.tile([128, cfg.c2], f32)
            nc.sync.dma_start(b2_s[:], b2r[:, :])
            iota_s = cpool.tile([128, 128], f32)
            nc.sync.dma_start(iota_s[:], iota_in[:, :])
            zf = cpool.tile([128, 128], f32)
            nc.vector.memset(zf[:], 0.0)
            zb = cpool.tile([128, 128], bf16)
            nc.vector.memset(zb[:], 0.0)

            # ---- phase 0: zero accumulators, D tables, table zero-rows
            # one DMA covers many 128-row tiles via a step-0 repeat of the
            # (fully initialized) zero tile
            def zfill(t, nr, zt):
                o = 0
                while o < nr:
                    full = min(16, (nr - o) // 128)
                    if full:
                        nc.scalar.dma_start(
                            t[o:o + full * 128, :].rearrange(
                                "(a p) c -> p a c", p=128),
                            zt[:].rearrange("p (o c) -> p o c", o=1)
                                .to_broadcast([128, full, 128]))
                        o += full * 128
                    else:
                        nc.scalar.dma_start(t[o:nr, :], zt[:nr - o, :])
                        o = nr
            for name, t in accs.items():
                zfill(t, cfg.acc_rows, zf)
            for t in (d1t, d2t):
                zfill(t, cfg.acc_rows, zb)
            for t, nr in ((t2a, cfg.na), (t2b, cfg.nb)):
                zfill(t, nr, zb)
            nc.scalar.dma_start(t1a[cfg.na - 1:cfg.na, :], zb[0:1, :])
            nc.scalar.dma_start(t1b[cfg.nb - 1:cfg.nb, :], zb[0:1, :])

            # ---- phase 1: node pass over full x -> T1a/T1b (+ own -> D1)
            def node_pass1(src_dram, n_rows, dst_tables, d1_dst, pool, ppool):
                ntl = (n_rows + 127) // 128
                BN = 4
                if True:
                    def compute_tile(o, r, xt_ap, tb_ap):
                        xts = ppool.tile([cfg.in_ch, 128], f32, tag="xtp")
                        nc.tensor.transpose(out=xts[:, :r], in_=xt_ap[:r, :],
                                            identity=ident[:r, :r])
                        xT = pool.tile([cfg.in_ch, 128], f32, tag="xT")
                        nc.scalar.copy(out=xT[:, :r], in_=xts[:, :r])
                        hp = ppool.tile([128, 96], f32, tag="hp")
                        nc.tensor.matmul(out=hp[:r, :], lhsT=xT[:, :r],
                                         rhs=wc1_s[:], start=True, stop=True)
                        nc.scalar.copy(out=tb_ap[:r, 0:64], in_=hp[:r, 0:64])
                        nc.scalar.activation(out=tb_ap[:r, 64:96],
                                             in_=hp[:r, 64:96], func=Act.Exp)
                        nc.vector.memset(tb_ap[:r, 96:128], 0.0)

                    t = 0
                    while t < ntl:
                        o = t * 128
                        nb = min(BN, ntl - t)
                        rows = min(nb * 128, n_rows - o)
                        full = rows == nb * 128
                        # batched path only for full groups not crossing the
                        # half-table boundary
                        crosses = (d1_dst is None and o < cfg.half
                                   and o + rows > cfg.half)
                        if full and nb == BN and not crosses:
                            xt4 = pool.tile([128, BN, cfg.in_ch], f32, tag="xt4")
                            nc.sync.dma_start(
                                xt4[:],
                                src_dram[o:o + rows, :].rearrange(
                                    "(a p) c -> p a c", p=128))
                            tb4 = pool.tile([128, BN, 128], bf16, tag="tb4")
                            for j in range(BN):
                                compute_tile(o + j * 128, 128,
                                             xt4[:, j, :], tb4[:, j, :])
                            if d1_dst is None:
                                tab, toff = ((t1a, o) if o + rows <= cfg.half
                                             else (t1b, o - cfg.half))
                                nc.sync.dma_start(
                                    tab[toff:toff + rows, :].rearrange(
                                        "(a p) c -> p a c", p=128),
                                    tb4[:])
                            else:
                                nc.sync.dma_start(
                                    d1_dst[o:o + rows, 0:16].rearrange(
                                        "(a p) c -> p a c", p=128),
                                    tb4[:, :, 80:96])
                            t += BN
                        else:
                            r = min(128, n_rows - o)
                            xt = pool.tile([128, cfg.in_ch], f32, tag="xt")
                            nc.sync.dma_start(xt[:r, :], src_dram[o:o + r, :])
                            tb = pool.tile([128, 128], bf16, tag="tb")
                            compute_tile(o, r, xt[:], tb[:])
                            if d1_dst is None:
                                for (tab, goff, toff, rr) in dst_tables(o, r):
                                    nc.sync.dma_start(tab[toff:toff + rr, :],
                                                      tb[goff:goff + rr, :])
                            else:
                                nc.sync.dma_start(d1_dst[o:o + r, 0:16],
                                                  tb[:r, 80:96])
                            t += 1

            def t1_targets(o, r):
                out = []
                if o < cfg.half:
                    rr = min(r, cfg.half - o)
                    out.append((t1a, 0, o, rr))
                if o + r > cfg.half:
                    s = max(0, cfg.half - o)
                    out.append((t1b, s, o + s - cfg.half, r - s))
                return out



            # ---- edge pass (shared for both layers)
            def edge_pass(tabs, dtab, acc_a, acc_b, es_off, nh, hc, pool, ppool):
                dim = hc // nh
                G4 = 4
                scgA = scA // G4
                scgT = scn // G4
                if True:
                    for gi in range(scgT):
                        tab = tabs[0] if gi < scgA else tabs[1]
                        acc = acc_a if gi < scgA else acc_b
                        mt = pool.tile([128, 328], i16, tag="mt")
                        nc.sync.dma_start(mt[:], meta_in[gi, :, :])
                        dsl = mt[:, 320:328].bitcast(f32)
                        g = pool.tile([128, G4 * S, 128], bf16, tag="g")
                        for k in range(G4):
                            nc.gpsimd.dma_gather(
                                g[:, k * S:(k + 1) * S, :], tab[:, :],
                                mt[:, 64 * k:64 * k + 64],
                                128 * S, 128 * S, 128)
                        dt_ = pool.tile([128, G4, 128], bf16, tag="dt")
                        nc.gpsimd.dma_gather(
                            dt_[:], dtab[:, :], mt[:, 256:288],
                            G4 * 128, G4 * 128, 128)
                        gv = g[:].rearrange("p (k s) c -> p k s c", s=S)
                        m = pool.tile([128, G4, 128], bf16, tag="m")
                        for k in range(G4):
                            nc.vector.tensor_scalar(
                                out=m[:, k, :], in0=iota_s[:],
                                scalar1=dsl[:, k:k + 1], scalar2=None,
                                op0=Alu.is_equal)
                        u = pool.tile([128, G4, S, nh], bf16, tag="u")
                        v = pool.tile([128, G4, S, nh], bf16, tag="v")
                        r_ = pool.tile([128, G4 * S, hc + nh], bf16, tag="r")
                        rv = r_[:].rearrange("p (k s) c -> p k s c", s=S)
                        nc.vector.tensor_tensor(
                            out=u[:], in0=gv[:, :, :, es_off:es_off + nh],
                            in1=dt_[:].rearrange("p k (o c) -> p k o c", o=1)
                                [:, :, :, 0:nh].to_broadcast([128, G4, S, nh]),
                            op=Alu.mult)
                        nc.vector.tensor_tensor(
                            out=v[:], in0=gv[:, :, :, es_off + nh:es_off + 2 * nh],
                            in1=dt_[:].rearrange("p k (o c) -> p k o c", o=1)
                                [:, :, :, nh:2 * nh].to_broadcast([128, G4, S, nh]),
                            op=Alu.mult)
                        nc.vector.tensor_scalar(
                            out=v[:], in0=v[:], scalar1=1.0, scalar2=None,
                            op0=Alu.min)
                        nc.vector.tensor_tensor(
                            out=rv[:, :, :, hc:hc + nh], in0=u[:], in1=v[:],
                            op=Alu.max)
                        exb = rv[:, :, :, hc:hc + nh].rearrange(
                            "p k s (h o) -> p (k s) h o", o=1).to_broadcast(
                            [128, G4 * S, nh, dim])
                        nc.vector.tensor_tensor(
                            out=r_[:, :, 0:hc].rearrange(
                                "p c (h d) -> p c h d", d=dim),
                            in0=g[:, :, 0:hc].rearrange(
                                "p c (h d) -> p c h d", d=dim),
                            in1=exb, op=Alu.mult)
                        sout = pool.tile([128, G4, 128], f32, tag="sout")
                        nc.vector.memset(sout[:, :, hc + nh:128], 0.0)
                        for k in range(G4):
                            ps = ppool.tile([128, hc + nh], f32, tag="ps")
                            for s_ in range(S):
                                nc.tensor.matmul(out=ps[:], lhsT=m[:, k, :],
                                                 rhs=r_[:, k * S + s_, :],
                                                 start=(s_ == 0),
                                                 stop=(s_ == S - 1))
                            nc.scalar.copy(out=sout[:, k, 0:hc + nh],
                                           in_=ps[:])
                        nc.gpsimd.dma_scatter_add(
                            acc[:, :], sout[:], mt[:, 288:320],
                            G4 * 128, G4 * 128, 128)

            with tc.tile_pool(name="np1", bufs=6) as np1_pool, \
                 tc.tile_pool(name="np1p", bufs=2, space="PSUM") as np1_ppool, \
                 tc.tile_pool(name="ep1", bufs=6) as ep_pool, \
                 tc.tile_pool(name="ep1p", bufs=4, space="PSUM") as ep_ppool:
                node_pass1(x_own, cfg.rows_core, None, d1t, np1_pool, np1_ppool)
                node_pass1(x_in, cfg.n_nodes, t1_targets, None, np1_pool, np1_ppool)
                edge_pass((t1a, t1b), d1t, accs["a1A"], accs["a1B"], 64,
                          cfg.h1, cfg.c1, ep_pool, ep_ppool)


            # ---- phase 3: layer-2 node pass (local rows)
            # batched path for exactly-4-full-tile groups; leftover per-tile
            with tc.tile_pool(name="np2", bufs=4) as pool, \
                 tc.tile_pool(name="np2p", bufs=4, space="PSUM") as ppool:
                for g in range(otile // 4):
                    o = g * 512
                    ra = pool.tile([128, 4, 72], f32, tag="ra4")
                    rb = pool.tile([128, 4, 72], f32, tag="rb4")
                    nc.sync.dma_start(
                        ra[:], accs["a1A"][o:o + 512, 0:72]
                        .rearrange("(a p) c -> p a c", p=128))
                    nc.sync.dma_start(
                        rb[:], accs["a1B"][o:o + 512, 0:72]
                        .rearrange("(a p) c -> p a c", p=128))
                    nc.vector.tensor_tensor(out=ra[:], in0=ra[:], in1=rb[:],
                                            op=Alu.add)
                    den = pool.tile([128, 4, cfg.h1], f32, tag="den4")
                    nc.vector.tensor_scalar(out=den[:], in0=ra[:, :, 64:72],
                                            scalar1=EPS, scalar2=None,
                                            op0=Alu.add)
                    rec = pool.tile([128, 4, cfg.h1], f32, tag="rec4")
                    nc.vector.reciprocal(out=rec[:], in_=den[:])
                    h2 = pool.tile([128, 4, cfg.c1], f32, tag="h24")
                    nc.vector.tensor_tensor(
                        out=h2[:].rearrange("p a (h d) -> p a h d", d=cfg.d1),
                        in0=ra[:, :, 0:64].rearrange("p a (h d) -> p a h d",
                                                     d=cfg.d1),
                        in1=rec[:].rearrange("p a (h o) -> p a h o", o=1)
                            .to_broadcast([128, 4, cfg.h1, cfg.d1]),
                        op=Alu.mult)
                    nc.vector.tensor_tensor(
                        out=h2[:], in0=h2[:],
                        in1=b1_s[:, :].rearrange("p (o c) -> p o c", o=1)
                            .to_broadcast([128, 4, cfg.c1]), op=Alu.add)
                    t1_ = pool.tile([128, 4, cfg.c1], f32, tag="t14")
                    nc.vector.tensor_scalar(out=t1_[:], in0=h2[:], scalar1=0.0,
                                            scalar2=None, op0=Alu.min)
                    nc.scalar.activation(out=t1_[:], in_=t1_[:], func=Act.Exp)
                    nc.vector.tensor_scalar(out=h2[:], in0=h2[:], scalar1=0.0,
                                            scalar2=None, op0=Alu.max)
                    nc.vector.tensor_tensor(out=h2[:], in0=h2[:], in1=t1_[:],
                                            op=Alu.add)
                    nc.vector.tensor_scalar(out=h2[:], in0=h2[:], scalar1=-1.0,
                                            scalar2=None, op0=Alu.add)
                    ag = pool.tile([128, 4, 34], bf16, tag="ag4")
                    ex4 = pool.tile([128, 4, 4], bf16, tag="ex44")
                    for j in range(4):
                        hts = ppool.tile([cfg.c1, 128], f32, tag="hts")
                        nc.tensor.transpose(out=hts[:], in_=h2[:, j, :],
                                            identity=ident[:])
                        hT = pool.tile([cfg.c1, 128], f32, tag="hT")
                        nc.scalar.copy(out=hT[:], in_=hts[:])
                        p2 = ppool.tile([128, 36], f32, tag="p2")
                        nc.tensor.matmul(out=p2[:], lhsT=hT[:], rhs=wc2_s[:],
                                         start=True, stop=True)
                        nc.vector.tensor_copy(out=ag[:, j, 0:32],
                                              in_=p2[:, 0:32])
                        nc.scalar.activation(out=ex4[:, j, :],
                                             in_=p2[:, 32:36], func=Act.Exp)
                        nc.vector.tensor_copy(out=ag[:, j, 32:34],
                                              in_=ex4[:, j, 0:2])
                    nc.sync.dma_start(
                        agin[o:o + 512, :].rearrange("(a p) c -> p a c", p=128),
                        ag[:])
                    nc.sync.dma_start(
                        d2t[o:o + 512, 0:2].rearrange("(a p) c -> p a c", p=128),
                        ex4[:, :, 2:4])
                for t in range(4 * (otile // 4), otile):
                    o = t * 128
                    r = min(128, cfg.rows_core - o)
                    ra = pool.tile([128, 72], f32, tag="ra")
                    rb = pool.tile([128, 72], f32, tag="rb")
                    nc.sync.dma_start(ra[:r, :], accs["a1A"][o:o + r, 0:72])
                    nc.sync.dma_start(rb[:r, :], accs["a1B"][o:o + r, 0:72])
                    nc.vector.tensor_tensor(out=ra[:r, :], in0=ra[:r, :], in1=rb[:r, :],
                                            op=Alu.add)
                    den = pool.tile([128, cfg.h1], f32, tag="den")
                    nc.vector.tensor_scalar(out=den[:r, :], in0=ra[:r, 64:72],
                                            scalar1=EPS, scalar2=None, op0=Alu.add)
                    rec = pool.tile([128, cfg.h1], f32, tag="rec")
                    nc.vector.reciprocal(out=rec[:r, :], in_=den[:r, :])
                    h2 = pool.tile([128, cfg.c1], f32, tag="h2")
                    nc.vector.tensor_tensor(
                        out=h2[:r, :].rearrange("p (h d) -> p h d", d=cfg.d1),
                        in0=ra[:r, 0:64].rearrange("p (h d) -> p h d", d=cfg.d1),
                        in1=rec[:r, :].rearrange("p (h o) -> p h o", o=1)
                            .to_broadcast([r, cfg.h1, cfg.d1]),
                        op=Alu.mult)
                    nc.vector.tensor_tensor(out=h2[:r, :], in0=h2[:r, :],
                                            in1=b1_s[:r, :], op=Alu.add)
                    # ELU: max(x,0) + exp(min(x,0)) - 1
                    t1_ = pool.tile([128, cfg.c1], f32, tag="t1_")
                    nc.vector.tensor_scalar(out=t1_[:r, :], in0=h2[:r, :],
                                            scalar1=0.0, scalar2=None, op0=Alu.min)
                    nc.scalar.activation(out=t1_[:r, :], in_=t1_[:r, :], func=Act.Exp)
                    nc.vector.tensor_scalar(out=h2[:r, :], in0=h2[:r, :],
                                            scalar1=0.0, scalar2=None, op0=Alu.max)
                    nc.vector.tensor_tensor(out=h2[:r, :], in0=h2[:r, :],
                                            in1=t1_[:r, :], op=Alu.add)
                    nc.vector.tensor_scalar(out=h2[:r, :], in0=h2[:r, :],
                                            scalar1=-1.0, scalar2=None, op0=Alu.add)
                    hts = ppool.tile([cfg.c1, 128], f32, tag="hts")
                    nc.tensor.transpose(out=hts[:, :r], in_=h2[:r, :], identity=ident[:r, :r])
                    hT = pool.tile([cfg.c1, 128], f32, tag="hT")
                    nc.scalar.copy(out=hT[:, :r], in_=hts[:, :r])
                    p2 = ppool.tile([128, 36], f32, tag="p2")
                    nc.tensor.matmul(out=p2[:r, :], lhsT=hT[:, :r], rhs=wc2_s[:],
                                     start=True, stop=True)
                    ag = pool.tile([128, 34], bf16, tag="ag")
                    nc.vector.tensor_copy(out=ag[:r, 0:32], in_=p2[:r, 0:32])
                    ex4 = pool.tile([128, 4], bf16, tag="ex4")
                    nc.scalar.activation(out=ex4[:r, :], in_=p2[:r, 32:36], func=Act.Exp)
                    nc.vector.tensor_copy(out=ag[:r, 32:34], in_=ex4[:r, 0:2])
                    nc.sync.dma_start(agin[o:o + r, :], ag[:r, :])
                    nc.sync.dma_start(d2t[o:o + r, 0:2], ex4[:r, 2:4])

            tc.strict_bb_all_engine_barrier()

            # ---- phase 4: AllGather + restride into T2a/T2b
            if with_cc:
                nc.gpsimd.collective_compute(
                    "AllGather", Alu.bypass,
                    replica_groups=[list(range(cfg.ncores))],
                    ins=[agin[:, :]], outs=[agout[:, :]])
            tc.strict_bb_all_engine_barrier()
            nc.sync.dma_start(t2a[0:cfg.half, 0:34], agout[0:cfg.half, :])
            nc.sync.dma_start(t2b[0:cfg.n_nodes - cfg.half, 0:34],
                              agout[cfg.half:cfg.n_nodes, :])


            # ---- phase 5: layer-2 edge pass
            with tc.tile_pool(name="ep2", bufs=6) as ep_pool2, \
                 tc.tile_pool(name="ep2p", bufs=6, space="PSUM") as ep_ppool2:
                edge_pass((t2a, t2b), d2t, accs["a2A"], accs["a2B"], 32,
                          H2, cfg.c2, ep_pool2, ep_ppool2)


            # ---- phase 6: tail (normalize + bias + log_softmax), 4 tiles/step
            with tc.tile_pool(name="tl", bufs=4) as pool:
                t = 0
                while t < otile:
                    o = t * 128
                    nb = min(4, otile - t)
                    rows = min(nb * 128, cfg.rows_core - o)
                    if rows < nb * 128:
                        nb -= 1
                        rows = nb * 128
                    if nb >= 1:
                        ra = pool.tile([128, 4, 33], f32, tag="tra")
                        rb = pool.tile([128, 4, 33], f32, tag="trb")
                        nc.sync.dma_start(
                            ra[:, :nb, :], accs["a2A"][o:o + rows, 0:33]
                            .rearrange("(a p) c -> p a c", p=128))
                        nc.sync.dma_start(
                            rb[:, :nb, :], accs["a2B"][o:o + rows, 0:33]
                            .rearrange("(a p) c -> p a c", p=128))
                        nc.vector.tensor_tensor(out=ra[:, :nb, :],
                                                in0=ra[:, :nb, :],
                                                in1=rb[:, :nb, :], op=Alu.add)
                        den = pool.tile([128, 4], f32, tag="tden")
                        nc.vector.tensor_scalar(
                            out=den[:, :nb], in0=ra[:, :nb, 32], scalar1=EPS,
                            scalar2=None, op0=Alu.add)
                        rec = pool.tile([128, 4], f32, tag="trec")
                        nc.vector.reciprocal(out=rec[:, :nb], in_=den[:, :nb])
                        y = pool.tile([128, 4, 32], f32, tag="ty")
                        nc.vector.tensor_tensor(
                            out=y[:, :nb, :], in0=ra[:, :nb, 0:32],
                            in1=rec[:, :nb].rearrange("p (a o) -> p a o", o=1)
                                .to_broadcast([128, nb, 32]), op=Alu.mult)
                        nc.vector.tensor_tensor(
                            out=y[:, :nb, :], in0=y[:, :nb, :],
                            in1=b2_s[:, :].rearrange("p (o c) -> p o c", o=1)
                                .to_broadcast([128, nb, 32]), op=Alu.add)
                        mx = pool.tile([128, 4], f32, tag="tmx")
                        nc.vector.reduce_max(out=mx[:, :nb], in_=y[:, :nb, :],
                                             axis=mybir.AxisListType.X)
                        nc.vector.tensor_tensor(
                            out=y[:, :nb, :], in0=y[:, :nb, :],
                            in1=mx[:, :nb].rearrange("p (a o) -> p a o", o=1)
                                .to_broadcast([128, nb, 32]), op=Alu.subtract)
                        ey = pool.tile([128, 4, 32], f32, tag="tey")
                        nc.scalar.activation(out=ey[:, :nb, :], in_=y[:, :nb, :],
                                             func=Act.Exp)
                        sm = pool.tile([128, 4], f32, tag="tsm")
                        nc.vector.reduce_sum(out=sm[:, :nb], in_=ey[:, :nb, :],
                                             axis=mybir.AxisListType.X)
                        lg = pool.tile([128, 4], f32, tag="tlg")
                        nc.scalar.activation(out=lg[:, :nb], in_=sm[:, :nb],
                                             func=Act.Ln)
                        nc.vector.tensor_tensor(
                            out=y[:, :nb, :], in0=y[:, :nb, :],
                            in1=lg[:, :nb].rearrange("p (a o) -> p a o", o=1)
                                .to_broadcast([128, nb, 32]), op=Alu.subtract)
                        nc.sync.dma_start(
                            out_ext[o:o + rows, :]
                            .rearrange("(a p) c -> p a c", p=128),
                            y[:, :nb, :])
                        t += nb
                        continue
                    r = cfg.rows_core - o
                    ra1 = pool.tile([128, 33], f32, tag="tra1")
                    rb1 = pool.tile([128, 33], f32, tag="trb1")
                    nc.sync.dma_start(ra1[:r, :], accs["a2A"][o:o + r, 0:33])
                    nc.sync.dma_start(rb1[:r, :], accs["a2B"][o:o + r, 0:33])
                    nc.vector.tensor_tensor(out=ra1[:r, :], in0=ra1[:r, :],
                                            in1=rb1[:r, :], op=Alu.add)
                    den1 = pool.tile([128, 1], f32, tag="tden1")
                    nc.vector.tensor_scalar(out=den1[:r, :], in0=ra1[:r, 32:33],
                                            scalar1=EPS, scalar2=None, op0=Alu.add)
                    rec1 = pool.tile([128, 1], f32, tag="trec1")
                    nc.vector.reciprocal(out=rec1[:r, :], in_=den1[:r, :])
                    y1 = pool.tile([128, cfg.c2], f32, tag="ty1")
                    nc.vector.tensor_scalar(out=y1[:r, :], in0=ra1[:r, 0:32],
                                            scalar1=rec1[:r, :], scalar2=None,
                                            op0=Alu.mult)
                    nc.vector.tensor_tensor(out=y1[:r, :], in0=y1[:r, :],
                                            in1=b2_s[:r, :], op=Alu.add)
                    mx1 = pool.tile([128, 1], f32, tag="tmx1")
                    nc.vector.reduce_max(out=mx1[:r, :], in_=y1[:r, :],
                                         axis=mybir.AxisListType.X)
                    nc.vector.tensor_scalar(out=y1[:r, :], in0=y1[:r, :],
                                            scalar1=mx1[:r, :], scalar2=None,
                                            op0=Alu.subtract)
                    ey1 = pool.tile([128, cfg.c2], f32, tag="tey1")
                    nc.scalar.activation(out=ey1[:r, :], in_=y1[:r, :], func=Act.Exp)
                    sm1 = pool.tile([128, 1], f32, tag="tsm1")
                    nc.vector.reduce_sum(out=sm1[:r, :], in_=ey1[:r, :],
                                         axis=mybir.AxisListType.X)
                    lg1 = pool.tile([128, 1], f32, tag="tlg1")
                    nc.scalar.activation(out=lg1[:r, :], in_=sm1[:r, :], func=Act.Ln)
                    nc.vector.tensor_scalar(out=y1[:r, :], in0=y1[:r, :],
                                            scalar1=lg1[:r, :], scalar2=None,
                                            op0=Alu.subtract)
                    nc.sync.dma_start(out_ext[o:o + r, :], y1[:r, :])
                    t += 1

    if not nc.is_finalized():
        nc.finalize()
    return nc


# -------------------------------------------------------------------- weights
def host_weights(W1, a_src1, a_dst1, W2, a_src2, a_dst2, cfg: Cfg):
    # Ws[k,h] = sum_d W1[k, h*dim+d] * a_src1[h, d]
    W1r = W1.reshape(cfg.in_ch, cfg.h1, cfg.d1)
    Ws1 = np.einsum("khd,hd->kh", W1r, a_src1)
    Wd1 = np.einsum("khd,hd->kh", W1r, a_dst1)
    wc1 = np.concatenate([W1, Ws1, NEG * Ws1, Wd1, NEG * Wd1], axis=1).astype(np.float32)
    Ws2 = (W2 @ a_src2[0][:, None])  # [c1,1]
    Wd2 = (W2 @ a_dst2[0][:, None])
    wc2 = np.concatenate([W2, Ws2, NEG * Ws2, Wd2, NEG * Wd2], axis=1).astype(np.float32)
    return wc1, wc2


def host_inputs(x, edge_index, W1, a_src1, a_dst1, b1, W2, a_src2, a_dst2, b2,
                cfg: Cfg):
    n = cfg.n_nodes
    loops = np.arange(n, dtype=np.int64)
    src = np.concatenate([np.asarray(edge_index[0], np.int64), loops])
    dst = np.concatenate([np.asarray(edge_index[1], np.int64), loops])
    meta, dsl, scA, scB = build_plan(src, dst, cfg)
    # repack into 4-SC groups: gidx [128,256] | didx [128,32] | scat [128,32]
    scg = (scA + scB) // 4
    meta2 = np.zeros((cfg.ncores, scg, 128, 328), np.int16)
    dsl2 = np.zeros((cfg.ncores, scg, 128, 4), np.float32)
    for g in range(scg):
        for k in range(4):
            m = meta[:, 4 * g + k]
            meta2[:, g, :, 64 * k:64 * k + 64] = m[:, :, 0:64]
            meta2[:, g, :, 256 + 8 * k:256 + 8 * k + 8] = m[:, :, 64:72]
            meta2[:, g, :, 288 + 8 * k:288 + 8 * k + 8] = m[:, :, 72:80]
            dsl2[:, g, :, k] = dsl[:, 4 * g + k]
    meta2[:, :, :, 320:328] = dsl2.view(np.int16).reshape(
        cfg.ncores, scg, 128, 8)
    meta = meta2
    dsl = dsl2
    wc1, wc2 = host_weights(np.asarray(W1, np.float32), np.asarray(a_src1, np.float32),
                            np.asarray(a_dst1, np.float32), np.asarray(W2, np.float32),
                            np.asarray(a_src2, np.float32), np.asarray(a_dst2, np.float32),
                            cfg)
    b1rep = np.tile(np.asarray(b1, np.float32)[None, :], (128, 1))
    b2rep = np.tile(np.asarray(b2, np.float32)[None, :], (128, 1))
    iota = np.tile(np.arange(128, dtype=np.float32)[None, :], (128, 1))
    x = np.asarray(x, np.float32)
    in_maps = []
    for c in range(cfg.ncores):
        in_maps.append({
            "x_in": x,
            "x_own": np.ascontiguousarray(x[c * cfg.rows_core:(c + 1) * cfg.rows_core]),
            "wc1": wc1, "wc2": wc2, "b1r": b1rep, "b2r": b2rep,
            "iota_in": np.ascontiguousarray(iota),
            "meta_in": np.ascontiguousarray(meta[c]),
        })
    return in_maps, scA, scB


_CACHE = {}
LAST_RESULT = None


def kernel(**inputs) -> np.ndarray:
    cfg = Cfg()
    in_maps, scA, scB = host_inputs(cfg=cfg, **inputs)
    key = (scA, scB)
    if key not in _CACHE:
        _CACHE[key] = build_program(cfg, scA, scB)
    nc = _CACHE[key]
    from concourse.bass_utils import run_bass_kernel_spmd
    trace = bool(int(os.environ.get("GAT_TRACE", "0")))
    res = run_bass_kernel_spmd(nc, in_maps, list(range(cfg.ncores)), trace=trace)
    global LAST_RESULT
    LAST_RESULT = res
    outs = [np.asarray(res.results[c]["out_ext"], np.float32)
            for c in range(cfg.ncores)]
    return np.concatenate(outs, axis=0)


if __name__ == "__main__":
    pass



# revision 2
# speedup vs baseline: 2.6375x; 2.6375x over previous
"""GAT (2-layer, PyG-style) Trainium2 Bass kernel, 8-core SPMD.

Strategy (dst-sharded graph parallel):
  - Nodes sharded by dst range (6250/core). Edges (incl. self-loops) bucketed
    per core by dst, sorted, grouped into 8-slot sub-chunks (one dst each),
    128 sub-chunks = 1 super-chunk (SC). Two streams per core: src<HALF (A)
    and src>=HALF (B) so gather indices fit int16.
  - Node pass computes, per OWN node, h = x@W and the attention exponentials
    es=exp(s), es02=exp(0.2 s), ed=exp(d), ed02=exp(0.2 d) via a fused matmul
    x @ [W | Ws | 0.2Ws | Wd | 0.2Wd]; rows packed into 256B bf16 gather
    table rows.  Key identity (exact for leaky_relu slope 0.2):
        exp(leaky_relu(s+d)) = max(es*ed, min(es02*ed02, 1))
    which factorizes src/dst terms and avoids any per-edge transcendentals.
  - The per-core 6250-row table slice (+1 zero pad row) is AllGathered so the
    full 50008-row table lands on every core; src gathers read it directly
    (A half = rows 0:25004, B half = rows 25004:50008; pad rows double as the
    zero rows gathered for empty slots). This keeps per-call kernel inputs
    small (the PJRT arg-staging cost scales with input bytes, and dominated
    the old design that shipped full x to every core).
  - Edge pass per SC: dma_gather rows by src (h,es,es02), dma_gather one row
    per sub-chunk by dst (ed,ed02), compute ex and ex-weighted h rows, then a
    membership matmul (lhsT = one-hot of dst-slot built by is_equal against an
    iota tile) accumulates per-dst sums + denominators in PSUM; result rows are
    dma_scatter_add-ed (CCE add) into a per-layer DRAM accumulator.
  - Edge metadata ships deduplicated (the gather index lists are 16-partition
    wraps replicated 8x on chip, not on the host).
  - Tail applies softmax-denominator, bias and log_softmax.
"""

import math
import os
import sys

import numpy as np

sys.path.insert(0, "/opt/trn_rl_repo")

import ml_dtypes

BF16 = ml_dtypes.bfloat16

# ---------------------------------------------------------------- problem cfg
N_NODES = 50000
N_EDGES = 1600000
IN_CH = 128
C1 = 64  # heads1*dim1
H1 = 8
D1H = 8
C2 = 32
H2 = 1
NEG = 0.2
EPS = 1e-16
NCORES = 8
HALF = 25000  # src-half split so gather idx fits int16
S = 8  # slots per sub-chunk
SCP = 128  # sub-chunks per super-chunk


class Cfg:
    def __init__(self, n_nodes=N_NODES, in_ch=IN_CH, c1=C1, h1=H1, c2=C2,
                 ncores=NCORES, half=HALF):
        self.n_nodes = n_nodes
        self.in_ch = in_ch
        self.c1 = c1
        self.h1 = h1
        self.d1 = c1 // h1
        self.c2 = c2
        self.ncores = ncores
        self.half = half
        self.rows_core = n_nodes // ncores
        self.acc_rows = ((self.rows_core + 127) // 128) * 128 + 256  # + park
        self.park = self.acc_rows - 192
        self.slice_rows = self.rows_core + 1   # own table slice + zero pad row
        self.tab_rows = self.slice_rows * (ncores // 2)  # rows per src half
        self.zrow = self.rows_core  # local index of the pad row in a slice


# ------------------------------------------------------------- host edge plan
def _wrap_idx(idx, reps=16):
    """[n] -> wrapped [16, n/16] layout (pos i at [i%16, i//16])."""
    n = idx.shape[0]
    w = idx.reshape(n // 16, 16).T.copy()  # [16, n/16]
    return np.tile(w, (reps // 16, 1))


def build_plan(src, dst, cfg: Cfg):
    """Per-core, per-stream super-chunk plan. Returns meta [cores, SCN, 16, 80]
    int16 (wrapped idx lists), dsl [cores, SCN, 128] f32, and (scA, scB).

    Gather indices address the AllGathered table: global node g owned by core
    c (= g // rows_core) sits at slice-local row (g - c*rows_core) of slice c,
    i.e. half-local index rel = (g % HALF) + (g % HALF) // rows_core within
    its half; the pad row of the half's first slice (index cfg.zrow) is zero
    and is used for empty slots.
    """
    rows = cfg.rows_core
    half = cfg.half
    core_of = dst // rows
    per_core = []
    for c in range(cfg.ncores):
        m = core_of == c
        s_c = src[m].astype(np.int64)
        d_c = dst[m].astype(np.int64) - c * rows
        stream = (s_c >= half).astype(np.int64)
        order = np.lexsort((s_c, stream, d_c))
        s_c, d_c, stream = s_c[order], d_c[order], stream[order]
        per_core.append((s_c, d_c, stream))

    # sub-chunk lists per (core, stream)
    def subchunks(s_c, d_c, st_c, which):
        m = st_c == which
        s, d = s_c[m], d_c[m]
        if s.shape[0] == 0:
            return (np.zeros((0, S), np.int64), np.zeros((0,), np.int64))
        # group by dst (sorted); ranks within group
        chg = np.r_[True, d[1:] != d[:-1]]
        gid = np.cumsum(chg) - 1
        gstart = np.flatnonzero(chg)
        rank = np.arange(d.shape[0]) - gstart[gid]
        sub_l = rank // S          # sub-chunk index within group
        slot = rank % S
        gsub = np.zeros(gid.max() + 1, np.int64)
        np.maximum.at(gsub, gid, sub_l + 1)          # sub-chunks per group
        gsub_off = np.r_[0, np.cumsum(gsub)]
        subid = gsub_off[gid] + sub_l
        nsub = int(gsub_off[-1])
        src_slots = np.zeros((nsub, S), np.int64)    # pad -> zero row
        src_slots[:] = cfg.zrow
        sh = s - (0 if which == 0 else half)         # half-local node id
        rel = sh + sh // rows                        # skip pad rows
        src_slots[subid, slot] = rel
        sub_dst = np.repeat(d[gstart], gsub)         # dst_rel per sub-chunk
        return src_slots, sub_dst

    plans = []  # per core: list of SC dicts per stream
    maxsc = [0, 0]
    for c in range(cfg.ncores):
        s_c, d_c, st_c = per_core[c]
        streams = []
        for which in (0, 1):
            src_slots, sub_dst = subchunks(s_c, d_c, st_c, which)
            # pack whole dst-groups (consecutive equal sub_dst) into SCs <=128
            scs = []
            n = sub_dst.shape[0]
            i = 0
            cur = []  # list of (start, count) groups
            cur_n = 0
            while i < n:
                j = i
                while j < n and sub_dst[j] == sub_dst[i]:
                    j += 1
                g = j - i
                assert g <= SCP, "dst run too large for one super-chunk"
                if cur_n + g > SCP:
                    scs.append((cur, cur_n))
                    cur, cur_n = [], 0
                cur.append((i, g))
                cur_n += g
                i = j
            if cur_n:
                scs.append((cur, cur_n))
            streams.append((src_slots, sub_dst, scs))
            maxsc[which] = max(maxsc[which], len(scs))
        plans.append(streams)

    scA = ((maxsc[0] + 3) // 4) * 4
    scB = ((maxsc[1] + 3) // 4) * 4
    scn = scA + scB
    meta = np.zeros((cfg.ncores, scn, 16, 80), np.int16)
    dsl = np.full((cfg.ncores, scn, 128), -1.0, np.float32)
    for c in range(cfg.ncores):
        for which in (0, 1):
            src_slots, sub_dst, scs = plans[c][which]
            base = 0 if which == 0 else scA
            nsc = scA if which == 0 else scB
            for k in range(nsc):
                g_idx = np.full((128, S), cfg.zrow, np.int64)
                d_idx = np.full((128,), cfg.park, np.int64)
                dstslot = np.full((128,), -1.0, np.float64)
                scat = np.full((128,), 0, np.int64)
                scat[:] = cfg.park + np.arange(128) % 64
                if k < len(scs):
                    groups, _n = scs[k]
                    p = 0
                    sid = 0
                    for (gs, gc) in groups:
                        g_idx[p:p + gc] = src_slots[gs:gs + gc]
                        d_idx[p:p + gc] = sub_dst[gs]
                        dstslot[p:p + gc] = sid
                        scat[sid] = sub_dst[gs]
                        p += gc
                        sid += 1
                mrow = meta[c, base + k]
                # gather idx list: position i = slot*128 + p
                flat = g_idx.T.reshape(-1)  # [1024] pos-ordered
                mrow[:, 0:64] = _wrap_idx(flat.astype(np.int16))
                mrow[:, 64:72] = _wrap_idx(d_idx.astype(np.int16))
                mrow[:, 72:80] = _wrap_idx(scat.astype(np.int16))
                dsl[c, base + k] = dstslot.astype(np.float32)
    return meta, dsl, scA, scB


# --------------------------------------------------------------- bass program
def build_program(cfg: Cfg, scA, scB, with_cc=True):
    from concourse import bacc, bass, library_config, mybir, tile
    from concourse.masks import make_identity

    f32 = mybir.dt.float32
    bf16 = mybir.dt.bfloat16
    i16 = mybir.dt.int16
    Alu = mybir.AluOpType
    Act = mybir.ActivationFunctionType

    scn = scA + scB
    scg = scn // 4
    nc = bacc.Bacc(None, target_bir_lowering=False, debug=False)

    # ---- I/O (kept small: per-call PJRT arg staging scales with bytes)
    x_own = nc.dram_tensor("x_own", [cfg.rows_core, cfg.in_ch], bf16, kind="ExternalInput")
    wc1 = nc.dram_tensor("wc1", [cfg.in_ch, 96], f32, kind="ExternalInput")
    wc2 = nc.dram_tensor("wc2", [cfg.c1, 36], f32, kind="ExternalInput")
    b1c = nc.dram_tensor("b1c", [1, cfg.c1], f32, kind="ExternalInput")
    b2c = nc.dram_tensor("b2c", [1, cfg.c2], f32, kind="ExternalInput")
    iota1 = nc.dram_tensor("iota1", [1, 128], f32, kind="ExternalInput")
    meta16_in = nc.dram_tensor("meta16_in", [scg, 16, 320], i16, kind="ExternalInput")
    dsl_in = nc.dram_tensor("dsl_in", [scg, 128, 8], i16, kind="ExternalInput")
    out_ext = nc.dram_tensor("out_ext", [cfg.rows_core, cfg.c2], f32, kind="ExternalOutput")

    # ---- internal DRAM
    ag1in = nc.dram_tensor("ag1in", [cfg.slice_rows, 128], bf16)
    ag2in = nc.dram_tensor("ag2in", [cfg.slice_rows, 128], bf16)
    ago1 = nc.dram_tensor("ago1", [cfg.slice_rows * cfg.ncores, 128], bf16,
                          addr_space="Shared")
    ago2 = nc.dram_tensor("ago2", [cfg.slice_rows * cfg.ncores, 128], bf16,
                          addr_space="Shared")
    d1t = nc.dram_tensor("d1t", [cfg.acc_rows, 128], bf16)
    d2t = nc.dram_tensor("d2t", [cfg.acc_rows, 128], bf16)
    acc1 = nc.dram_tensor("acc1", [cfg.acc_rows, 128], f32)
    acc2 = nc.dram_tensor("acc2", [cfg.acc_rows, 128], f32)

    otile = (cfg.rows_core + 127) // 128

    with tile.TileContext(nc) as tc:
        nc.gpsimd.load_library(library_config.mlp)
        with tc.tile_pool(name="const", bufs=1) as cpool:
            ident = cpool.tile([128, 128], f32)
            make_identity(nc, ident[:])
            wc1_s = cpool.tile([cfg.in_ch, 96], f32)
            nc.sync.dma_start(wc1_s[:], wc1[:, :])
            wc2_s = cpool.tile([cfg.c1, 36], f32)
            nc.sync.dma_start(wc2_s[:], wc2[:, :])
            b1_s = cpool.tile([128, cfg.c1], f32)
            nc.sync.dma_start(b1_s[0:1, :], b1c[:, :])
            nc.gpsimd.partition_broadcast(b1_s[:], b1_s[0:1, :])
            b2_s = cpool.tile([128, cfg.c2], f32)
            nc.sync.dma_start(b2_s[0:1, :], b2c[:, :])
            nc.gpsimd.partition_broadcast(b2_s[:], b2_s[0:1, :])
            iota_s = cpool.tile([128, 128], f32)
            nc.sync.dma_start(iota_s[0:1, :], iota1[:, :])
            nc.gpsimd.partition_broadcast(iota_s[:], iota_s[0:1, :])
            zf = cpool.tile([128, 128], f32)
            nc.vector.memset(zf[:], 0.0)
            zb = cpool.tile([128, 128], bf16)
            nc.vector.memset(zb[:], 0.0)

            # ---- phase 0: zero accumulators + D tables
            # one DMA covers many 128-row tiles via a step-0 repeat of the
            # (fully initialized) zero tile
            def zfill(t, nr, zt):
                o = 0
                while o < nr:
                    full = min(16, (nr - o) // 128)
                    if full:
                        nc.scalar.dma_start(
                            t[o:o + full * 128, :].rearrange(
                                "(a p) c -> p a c", p=128),
                            zt[:].rearrange("p (o c) -> p o c", o=1)
                                .to_broadcast([128, full, 128]))
                        o += full * 128
                    else:
                        nc.scalar.dma_start(t[o:nr, :], zt[:nr - o, :])
                        o = nr
            zfill(acc1, cfg.acc_rows, zf)
            zfill(acc2, cfg.acc_rows, zf)
            for t in (d1t, d2t):
                zfill(t, cfg.acc_rows, zb)
            # pad rows of the AllGather slices (gathered for empty slots)
            nc.scalar.dma_start(ag1in[cfg.zrow:cfg.zrow + 1, :], zb[0:1, :])
            nc.scalar.dma_start(ag2in[cfg.zrow:cfg.zrow + 1, :], zb[0:1, :])

            # ---- phase 1: node pass over OWN rows -> ag1in slice (+ d1 stats)
            def node_pass1(pool, ppool):
                BN = 4

                def compute_tile(r, xt_ap, tb_ap):
                    xts = ppool.tile([cfg.in_ch, 128], f32, tag="xtp")
                    nc.tensor.transpose(out=xts[:, :r], in_=xt_ap[:r, :],
                                        identity=ident[:r, :r])
                    xT = pool.tile([cfg.in_ch, 128], f32, tag="xT")
                    nc.scalar.copy(out=xT[:, :r], in_=xts[:, :r])
                    hp = ppool.tile([128, 96], f32, tag="hp")
                    nc.tensor.matmul(out=hp[:r, :], lhsT=xT[:, :r],
                                     rhs=wc1_s[:], start=True, stop=True)
                    nc.scalar.copy(out=tb_ap[:r, 0:64], in_=hp[:r, 0:64])
                    nc.scalar.activation(out=tb_ap[:r, 64:96],
                                         in_=hp[:r, 64:96], func=Act.Exp)
                    nc.vector.memset(tb_ap[:r, 96:128], 0.0)

                t = 0
                while t < otile:
                    o = t * 128
                    nb = min(BN, otile - t)
                    rows = min(nb * 128, cfg.rows_core - o)
                    if nb == BN and rows == nb * 128:
                        xtb = pool.tile([128, BN, cfg.in_ch], bf16, tag="xtb")
                        nc.sync.dma_start(
                            xtb[:],
                            x_own[o:o + rows, :].rearrange(
                                "(a p) c -> p a c", p=128))
                        xt4 = pool.tile([128, BN, cfg.in_ch], f32, tag="xt4")
                        nc.scalar.copy(out=xt4[:], in_=xtb[:])
                        tb4 = pool.tile([128, BN, 128], bf16, tag="tb4")
                        for j in range(BN):
                            compute_tile(128, xt4[:, j, :], tb4[:, j, :])
                        nc.sync.dma_start(
                            ag1in[o:o + rows, :].rearrange(
                                "(a p) c -> p a c", p=128),
                            tb4[:])
                        nc.sync.dma_start(
                            d1t[o:o + rows, 0:16].rearrange(
                                "(a p) c -> p a c", p=128),
                            tb4[:, :, 80:96])
                        t += BN
                    else:
                        r = min(128, cfg.rows_core - o)
                        xtb = pool.tile([128, cfg.in_ch], bf16, tag="xtb1")
                        nc.sync.dma_start(xtb[:r, :], x_own[o:o + r, :])
                        xt = pool.tile([128, cfg.in_ch], f32, tag="xt1")
                        nc.scalar.copy(out=xt[:r, :], in_=xtb[:r, :])
                        tb = pool.tile([128, 128], bf16, tag="tb1")
                        compute_tile(r, xt[:], tb[:])
                        nc.sync.dma_start(ag1in[o:o + r, :], tb[:r, :])
                        nc.sync.dma_start(d1t[o:o + r, 0:16], tb[:r, 80:96])
                        t += 1

            # ---- edge pass (shared for both layers)
            def edge_pass(tab_a, tab_b, dtab, acc, es_off, nh, hc, pool, ppool):
                dim = hc // nh
                G4 = 4
                scgA = scA // G4
                scgT = scn // G4
                for gi in range(scgT):
                    tab = tab_a if gi < scgA else tab_b
                    mt = pool.tile([128, 328], i16, tag="mt")
                    for r8 in range(8):
                        nc.sync.dma_start(mt[16 * r8:16 * r8 + 16, 0:320],
                                          meta16_in[gi, :, :])
                    nc.sync.dma_start(mt[:, 320:328], dsl_in[gi, :, :])
                    dsl = mt[:, 320:328].bitcast(f32)
                    g = pool.tile([128, G4 * S, 128], bf16, tag="g")
                    for k in range(G4):
                        nc.gpsimd.dma_gather(
                            g[:, k * S:(k + 1) * S, :], tab,
                            mt[:, 64 * k:64 * k + 64],
                            128 * S, 128 * S, 128)
                    dt_ = pool.tile([128, G4, 128], bf16, tag="dt")
                    nc.gpsimd.dma_gather(
                        dt_[:], dtab[:, :], mt[:, 256:288],
                        G4 * 128, G4 * 128, 128)
                    gv = g[:].rearrange("p (k s) c -> p k s c", s=S)
                    m = pool.tile([128, G4, 128], bf16, tag="m")
                    for k in range(G4):
                        nc.vector.tensor_scalar(
                            out=m[:, k, :], in0=iota_s[:],
                            scalar1=dsl[:, k:k + 1], scalar2=None,
                            op0=Alu.is_equal)
                    u = pool.tile([128, G4, S, nh], bf16, tag="u")
                    v = pool.tile([128, G4, S, nh], bf16, tag="v")
                    r_ = pool.tile([128, G4 * S, hc + nh], bf16, tag="r")
                    rv = r_[:].rearrange("p (k s) c -> p k s c", s=S)
                    nc.vector.tensor_tensor(
                        out=u[:], in0=gv[:, :, :, es_off:es_off + nh],
                        in1=dt_[:].rearrange("p k (o c) -> p k o c", o=1)
                            [:, :, :, 0:nh].to_broadcast([128, G4, S, nh]),
                        op=Alu.mult)
                    nc.vector.tensor_tensor(
                        out=v[:], in0=gv[:, :, :, es_off + nh:es_off + 2 * nh],
                        in1=dt_[:].rearrange("p k (o c) -> p k o c", o=1)
                            [:, :, :, nh:2 * nh].to_broadcast([128, G4, S, nh]),
                        op=Alu.mult)
                    nc.vector.tensor_scalar(
                        out=v[:], in0=v[:], scalar1=1.0, scalar2=None,
                        op0=Alu.min)
                    nc.vector.tensor_tensor(
                        out=rv[:, :, :, hc:hc + nh], in0=u[:], in1=v[:],
                        op=Alu.max)
                    exb = rv[:, :, :, hc:hc + nh].rearrange(
                        "p k s (h o) -> p (k s) h o", o=1).to_broadcast(
                        [128, G4 * S, nh, dim])
                    nc.vector.tensor_tensor(
                        out=r_[:, :, 0:hc].rearrange(
                            "p c (h d) -> p c h d", d=dim),
                        in0=g[:, :, 0:hc].rearrange(
                            "p c (h d) -> p c h d", d=dim),
                        in1=exb, op=Alu.mult)
                    sout = pool.tile([128, G4, 128], f32, tag="sout")
                    nc.vector.memset(sout[:, :, hc + nh:128], 0.0)
                    for k in range(G4):
                        ps = ppool.tile([128, hc + nh], f32, tag="ps")
                        for s_ in range(S):
                            nc.tensor.matmul(out=ps[:], lhsT=m[:, k, :],
                                             rhs=r_[:, k * S + s_, :],
                                             start=(s_ == 0),
                                             stop=(s_ == S - 1))
                        nc.scalar.copy(out=sout[:, k, 0:hc + nh],
                                       in_=ps[:])
                    nc.gpsimd.dma_scatter_add(
                        acc[:, :], sout[:], mt[:, 288:320],
                        G4 * 128, G4 * 128, 128)

            with tc.tile_pool(name="np1", bufs=6) as np1_pool, \
                 tc.tile_pool(name="np1p", bufs=2, space="PSUM") as np1_ppool:
                node_pass1(np1_pool, np1_ppool)

            # ---- AllGather the layer-1 table slices
            tc.strict_bb_all_engine_barrier()
            if with_cc:
                nc.gpsimd.collective_compute(
                    "AllGather", mybir.AluOpType.bypass,
                    replica_groups=[list(range(cfg.ncores))],
                    ins=[ag1in[:, :]], outs=[ago1[:, :]])
            tc.strict_bb_all_engine_barrier()

            # ---- phase 2: layer-1 edge pass
            with tc.tile_pool(name="ep1", bufs=6) as ep_pool, \
                 tc.tile_pool(name="ep1p", bufs=4, space="PSUM") as ep_ppool:
                edge_pass(ago1[0:cfg.tab_rows, :],
                          ago1[cfg.tab_rows:2 * cfg.tab_rows, :],
                          d1t, acc1, 64, cfg.h1, cfg.c1, ep_pool, ep_ppool)

            # ---- phase 3: layer-2 node pass (local rows)
            # batched path for exactly-4-full-tile groups; leftover per-tile
            with tc.tile_pool(name="np2", bufs=4) as pool, \
                 tc.tile_pool(name="np2p", bufs=4, space="PSUM") as ppool:
                for g in range(otile // 4):
                    o = g * 512
                    ra = pool.tile([128, 4, 72], f32, tag="ra4")
                    nc.sync.dma_start(
                        ra[:], acc1[o:o + 512, 0:72]
                        .rearrange("(a p) c -> p a c", p=128))
                    den = pool.tile([128, 4, cfg.h1], f32, tag="den4")
                    nc.vector.tensor_scalar(out=den[:], in0=ra[:, :, 64:72],
                                            scalar1=EPS, scalar2=None,
                                            op0=Alu.add)
                    rec = pool.tile([128, 4, cfg.h1], f32, tag="rec4")
                    nc.vector.reciprocal(out=rec[:], in_=den[:])
                    h2 = pool.tile([128, 4, cfg.c1], f32, tag="h24")
                    nc.vector.tensor_tensor(
                        out=h2[:].rearrange("p a (h d) -> p a h d", d=cfg.d1),
                        in0=ra[:, :, 0:64].rearrange("p a (h d) -> p a h d",
                                                     d=cfg.d1),
                        in1=rec[:].rearrange("p a (h o) -> p a h o", o=1)
                            .to_broadcast([128, 4, cfg.h1, cfg.d1]),
                        op=Alu.mult)
                    nc.vector.tensor_tensor(
                        out=h2[:], in0=h2[:],
                        in1=b1_s[:, :].rearrange("p (o c) -> p o c", o=1)
                            .to_broadcast([128, 4, cfg.c1]), op=Alu.add)
                    t1_ = pool.tile([128, 4, cfg.c1], f32, tag="t14")
                    nc.vector.tensor_scalar(out=t1_[:], in0=h2[:], scalar1=0.0,
                                            scalar2=None, op0=Alu.min)
                    nc.scalar.activation(out=t1_[:], in_=t1_[:], func=Act.Exp)
                    nc.vector.tensor_scalar(out=h2[:], in0=h2[:], scalar1=0.0,
                                            scalar2=None, op0=Alu.max)
                    nc.vector.tensor_tensor(out=h2[:], in0=h2[:], in1=t1_[:],
                                            op=Alu.add)
                    nc.vector.tensor_scalar(out=h2[:], in0=h2[:], scalar1=-1.0,
                                            scalar2=None, op0=Alu.add)
                    ag = pool.tile([128, 4, 128], bf16, tag="ag4")
                    nc.vector.memset(ag[:, :, 34:128], 0.0)
                    ex4 = pool.tile([128, 4, 4], bf16, tag="ex44")
                    for j in range(4):
                        hts = ppool.tile([cfg.c1, 128], f32, tag="hts")
                        nc.tensor.transpose(out=hts[:], in_=h2[:, j, :],
                                            identity=ident[:])
                        hT = pool.tile([cfg.c1, 128], f32, tag="hT")
                        nc.scalar.copy(out=hT[:], in_=hts[:])
                        p2 = ppool.tile([128, 36], f32, tag="p2")
                        nc.tensor.matmul(out=p2[:], lhsT=hT[:], rhs=wc2_s[:],
                                         start=True, stop=True)
                        nc.vector.tensor_copy(out=ag[:, j, 0:32],
                                              in_=p2[:, 0:32])
                        nc.scalar.activation(out=ex4[:, j, :],
                                             in_=p2[:, 32:36], func=Act.Exp)
                        nc.vector.tensor_copy(out=ag[:, j, 32:34],
                                              in_=ex4[:, j, 0:2])
                    nc.sync.dma_start(
                        ag2in[o:o + 512, :].rearrange("(a p) c -> p a c", p=128),
                        ag[:])
                    nc.sync.dma_start(
                        d2t[o:o + 512, 0:2].rearrange("(a p) c -> p a c", p=128),
                        ex4[:, :, 2:4])
                for t in range(4 * (otile // 4), otile):
                    o = t * 128
                    r = min(128, cfg.rows_core - o)
                    ra = pool.tile([128, 72], f32, tag="ra")
                    nc.sync.dma_start(ra[:r, :], acc1[o:o + r, 0:72])
                    den = pool.tile([128, cfg.h1], f32, tag="den")
                    nc.vector.tensor_scalar(out=den[:r, :], in0=ra[:r, 64:72],
                                            scalar1=EPS, scalar2=None, op0=Alu.add)
                    rec = pool.tile([128, cfg.h1], f32, tag="rec")
                    nc.vector.reciprocal(out=rec[:r, :], in_=den[:r, :])
                    h2 = pool.tile([128, cfg.c1], f32, tag="h2")
                    nc.vector.tensor_tensor(
                        out=h2[:r, :].rearrange("p (h d) -> p h d", d=cfg.d1),
                        in0=ra[:r, 0:64].rearrange("p (h d) -> p h d", d=cfg.d1),
                        in1=rec[:r, :].rearrange("p (h o) -> p h o", o=1)
                            .to_broadcast([r, cfg.h1, cfg.d1]),
                        op=Alu.mult)
                    nc.vector.tensor_tensor(out=h2[:r, :], in0=h2[:r, :],
                                            in1=b1_s[:r, :], op=Alu.add)
                    # ELU: max(x,0) + exp(min(x,0)) - 1
                    t1_ = pool.tile([128, cfg.c1], f32, tag="t1_")
                    nc.vector.tensor_scalar(out=t1_[:r, :], in0=h2[:r, :],
                                            scalar1=0.0, scalar2=None, op0=Alu.min)
                    nc.scalar.activation(out=t1_[:r, :], in_=t1_[:r, :], func=Act.Exp)
                    nc.vector.tensor_scalar(out=h2[:r, :], in0=h2[:r, :],
                                            scalar1=0.0, scalar2=None, op0=Alu.max)
                    nc.vector.tensor_tensor(out=h2[:r, :], in0=h2[:r, :],
                                            in1=t1_[:r, :], op=Alu.add)
                    nc.vector.tensor_scalar(out=h2[:r, :], in0=h2[:r, :],
                                            scalar1=-1.0, scalar2=None, op0=Alu.add)
                    hts = ppool.tile([cfg.c1, 128], f32, tag="hts")
                    nc.tensor.transpose(out=hts[:, :r], in_=h2[:r, :], identity=ident[:r, :r])
                    hT = pool.tile([cfg.c1, 128], f32, tag="hT")
                    nc.scalar.copy(out=hT[:, :r], in_=hts[:, :r])
                    p2 = ppool.tile([128, 36], f32, tag="p2")
                    nc.tensor.matmul(out=p2[:r, :], lhsT=hT[:, :r], rhs=wc2_s[:],
                                     start=True, stop=True)
                    ag = pool.tile([128, 128], bf16, tag="ag")
                    nc.vector.memset(ag[:r, 34:128], 0.0)
                    nc.vector.tensor_copy(out=ag[:r, 0:32], in_=p2[:r, 0:32])
                    ex4 = pool.tile([128, 4], bf16, tag="ex4")
                    nc.scalar.activation(out=ex4[:r, :], in_=p2[:r, 32:36], func=Act.Exp)
                    nc.vector.tensor_copy(out=ag[:r, 32:34], in_=ex4[:r, 0:2])
                    nc.sync.dma_start(ag2in[o:o + r, :], ag[:r, :])
                    nc.sync.dma_start(d2t[o:o + r, 0:2], ex4[:r, 2:4])

            # ---- phase 4: AllGather the layer-2 table slices
            tc.strict_bb_all_engine_barrier()
            if with_cc:
                nc.gpsimd.collective_compute(
                    "AllGather", mybir.AluOpType.bypass,
                    replica_groups=[list(range(cfg.ncores))],
                    ins=[ag2in[:, :]], outs=[ago2[:, :]])
            tc.strict_bb_all_engine_barrier()

            # ---- phase 5: layer-2 edge pass
            with tc.tile_pool(name="ep2", bufs=6) as ep_pool2, \
                 tc.tile_pool(name="ep2p", bufs=6, space="PSUM") as ep_ppool2:
                edge_pass(ago2[0:cfg.tab_rows, :],
                          ago2[cfg.tab_rows:2 * cfg.tab_rows, :],
                          d2t, acc2, 32, H2, cfg.c2, ep_pool2, ep_ppool2)

            # ---- phase 6: tail (normalize + bias + log_softmax), 4 tiles/step
            with tc.tile_pool(name="tl", bufs=4) as pool:
                t = 0
                while t < otile:
                    o = t * 128
                    nb = min(4, otile - t)
                    rows = min(nb * 128, cfg.rows_core - o)
                    if rows < nb * 128:
                        nb -= 1
                        rows = nb * 128
                    if nb >= 1:
                        ra = pool.tile([128, 4, 33], f32, tag="tra")
                        nc.sync.dma_start(
                            ra[:, :nb, :], acc2[o:o + rows, 0:33]
                            .rearrange("(a p) c -> p a c", p=128))
                        den = pool.tile([128, 4], f32, tag="tden")
                        nc.vector.tensor_scalar(
                            out=den[:, :nb], in0=ra[:, :nb, 32], scalar1=EPS,
                            scalar2=None, op0=Alu.add)
                        rec = pool.tile([128, 4], f32, tag="trec")
                        nc.vector.reciprocal(out=rec[:, :nb], in_=den[:, :nb])
                        y = pool.tile([128, 4, 32], f32, tag="ty")
                        nc.vector.tensor_tensor(
                            out=y[:, :nb, :], in0=ra[:, :nb, 0:32],
                            in1=rec[:, :nb].rearrange("p (a o) -> p a o", o=1)
                                .to_broadcast([128, nb, 32]), op=Alu.mult)
                        nc.vector.tensor_tensor(
                            out=y[:, :nb, :], in0=y[:, :nb, :],
                            in1=b2_s[:, :].rearrange("p (o c) -> p o c", o=1)
                                .to_broadcast([128, nb, 32]), op=Alu.add)
                        mx = pool.tile([128, 4], f32, tag="tmx")
                        nc.vector.reduce_max(out=mx[:, :nb], in_=y[:, :nb, :],
                                             axis=mybir.AxisListType.X)
                        nc.vector.tensor_tensor(
                            out=y[:, :nb, :], in0=y[:, :nb, :],
                            in1=mx[:, :nb].rearrange("p (a o) -> p a o", o=1)
                                .to_broadcast([128, nb, 32]), op=Alu.subtract)
                        ey = pool.tile([128, 4, 32], f32, tag="tey")
                        nc.scalar.activation(out=ey[:, :nb, :], in_=y[:, :nb, :],
                                             func=Act.Exp)
                        sm = pool.tile([128, 4], f32, tag="tsm")
                        nc.vector.reduce_sum(out=sm[:, :nb], in_=ey[:, :nb, :],
                                             axis=mybir.AxisListType.X)
                        lg = pool.tile([128, 4], f32, tag="tlg")
                        nc.scalar.activation(out=lg[:, :nb], in_=sm[:, :nb],
                                             func=Act.Ln)
                        nc.vector.tensor_tensor(
                            out=y[:, :nb, :], in0=y[:, :nb, :],
                            in1=lg[:, :nb].rearrange("p (a o) -> p a o", o=1)
                                .to_broadcast([128, nb, 32]), op=Alu.subtract)
                        nc.sync.dma_start(
                            out_ext[o:o + rows, :]
                            .rearrange("(a p) c -> p a c", p=128),
                            y[:, :nb, :])
                        t += nb
                        continue
                    r = cfg.rows_core - o
                    ra1 = pool.tile([128, 33], f32, tag="tra1")
                    nc.sync.dma_start(ra1[:r, :], acc2[o:o + r, 0:33])
                    den1 = pool.tile([128, 1], f32, tag="tden1")
                    nc.vector.tensor_scalar(out=den1[:r, :], in0=ra1[:r, 32:33],
                                            scalar1=EPS, scalar2=None, op0=Alu.add)
                    rec1 = pool.tile([128, 1], f32, tag="trec1")
                    nc.vector.reciprocal(out=rec1[:r, :], in_=den1[:r, :])
                    y1 = pool.tile([128, cfg.c2], f32, tag="ty1")
                    nc.vector.tensor_scalar(out=y1[:r, :], in0=ra1[:r, 0:32],
                                            scalar1=rec1[:r, :], scalar2=None,
                                            op0=Alu.mult)
                    nc.vector.tensor_tensor(out=y1[:r, :], in0=y1[:r, :],
                                            in1=b2_s[:r, :], op=Alu.add)
                    mx1 = pool.tile([128, 1], f32, tag="tmx1")
                    nc.vector.reduce_max(out=mx1[:r, :], in_=y1[:r, :],
                                         axis=mybir.AxisListType.X)
                    nc.vector.tensor_scalar(out=y1[:r, :], in0=y1[:r, :],
                                            scalar1=mx1[:r, :], scalar2=None,
                                            op0=Alu.subtract)
                    ey1 = pool.tile([128, cfg.c2], f32, tag="tey1")
                    nc.scalar.activation(out=ey1[:r, :], in_=y1[:r, :], func=Act.Exp)
                    sm1 = pool.tile([128, 1], f32, tag="tsm1")
                    nc.vector.reduce_sum(out=sm1[:r, :], in_=ey1[:r, :],
                                         axis=mybir.AxisListType.X)
                    lg1 = pool.tile([128, 1], f32, tag="tlg1")
                    nc.scalar.activation(out=lg1[:r, :], in_=sm1[:r, :], func=Act.Ln)
                    nc.vector.tensor_scalar(out=y1[:r, :], in0=y1[:r, :],
                                            scalar1=lg1[:r, :], scalar2=None,
                                            op0=Alu.subtract)
                    nc.sync.dma_start(out_ext[o:o + r, :], y1[:r, :])
                    t += 1

    if not nc.is_finalized():
        nc.finalize()
    return nc


# -------------------------------------------------------------------- weights
def host_weights(W1, a_src1, a_dst1, W2, a_src2, a_dst2, cfg: Cfg):
    # Ws[k,h] = sum_d W1[k, h*dim+d] * a_src1[h, d]
    W1r = W1.reshape(cfg.in_ch, cfg.h1, cfg.d1)
    Ws1 = np.einsum("khd,hd->kh", W1r, a_src1)
    Wd1 = np.einsum("khd,hd->kh", W1r, a_dst1)
    wc1 = np.concatenate([W1, Ws1, NEG * Ws1, Wd1, NEG * Wd1], axis=1).astype(np.float32)
    Ws2 = (W2 @ a_src2[0][:, None])  # [c1,1]
    Wd2 = (W2 @ a_dst2[0][:, None])
    wc2 = np.concatenate([W2, Ws2, NEG * Ws2, Wd2, NEG * Wd2], axis=1).astype(np.float32)
    return wc1, wc2


def host_inputs(x, edge_index, W1, a_src1, a_dst1, b1, W2, a_src2, a_dst2, b2,
                cfg: Cfg):
    n = cfg.n_nodes
    loops = np.arange(n, dtype=np.int64)
    src = np.concatenate([np.asarray(edge_index[0], np.int64), loops])
    dst = np.concatenate([np.asarray(edge_index[1], np.int64), loops])
    meta, dsl, scA, scB = build_plan(src, dst, cfg)
    # repack into 4-SC groups:
    #   meta16 [scg,16,320] = gidx [16,256] | didx [16,32] | scat [16,32]
    #   dsl16  [scg,128,8]  = dstslot f32 x4 viewed as int16
    scg = (scA + scB) // 4
    meta16 = np.zeros((cfg.ncores, scg, 16, 320), np.int16)
    dsl16 = np.zeros((cfg.ncores, scg, 128, 8), np.int16)
    for g in range(scg):
        for k in range(4):
            m = meta[:, 4 * g + k]
            meta16[:, g, :, 64 * k:64 * k + 64] = m[:, :, 0:64]
            meta16[:, g, :, 256 + 8 * k:256 + 8 * k + 8] = m[:, :, 64:72]
            meta16[:, g, :, 288 + 8 * k:288 + 8 * k + 8] = m[:, :, 72:80]
            dsl16[:, g, :, 2 * k:2 * k + 2] = (
                dsl[:, 4 * g + k].astype(np.float32).view(np.int16)
                .reshape(cfg.ncores, 128, 2))
    wc1, wc2 = host_weights(np.asarray(W1, np.float32), np.asarray(a_src1, np.float32),
                            np.asarray(a_dst1, np.float32), np.asarray(W2, np.float32),
                            np.asarray(a_src2, np.float32), np.asarray(a_dst2, np.float32),
                            cfg)
    iota = np.arange(128, dtype=np.float32)[None, :]
    x = np.asarray(x, np.float32).astype(BF16)
    in_maps = []
    for c in range(cfg.ncores):
        in_maps.append({
            "x_own": np.ascontiguousarray(x[c * cfg.rows_core:(c + 1) * cfg.rows_core]),
            "wc1": wc1, "wc2": wc2,
            "b1c": np.asarray(b1, np.float32)[None, :],
            "b2c": np.asarray(b2, np.float32)[None, :],
            "iota1": np.ascontiguousarray(iota),
            "meta16_in": np.ascontiguousarray(meta16[c]),
            "dsl_in": np.ascontiguousarray(dsl16[c]),
        })
    return in_maps, scA, scB


_CACHE = {}
LAST_RESULT = None


def kernel(**inputs) -> np.ndarray:
    cfg = Cfg()
    in_maps, scA, scB = host_inputs(cfg=cfg, **inputs)
    key = (scA, scB)
    if key not in _CACHE:
        _CACHE[key] = build_program(cfg, scA, scB)
    nc = _CACHE[key]
    from concourse.bass_utils import run_bass_kernel_spmd
    trace = bool(int(os.environ.get("GAT_TRACE", "0")))
    res = run_bass_kernel_spmd(nc, in_maps, list(range(cfg.ncores)), trace=trace)
    global LAST_RESULT
    LAST_RESULT = res
    outs = [np.asarray(res.results[c]["out_ext"], np.float32)
            for c in range(cfg.ncores)]
    return np.concatenate(outs, axis=0)


if __name__ == "__main__":
    pass


# revision 5
# speedup vs baseline: 3.3717x; 1.2784x over previous
"""GAT (2-layer, PyG-style) Trainium2 Bass kernel, 8-core SPMD.

Strategy (dst-sharded graph parallel):
  - Nodes sharded by dst range (6250/core). Edges (incl. self-loops) bucketed
    per core by dst, sorted, grouped into 8-slot sub-chunks (one dst each),
    128 sub-chunks = 1 super-chunk (SC). Two streams per core: src<HALF (A)
    and src>=HALF (B) so gather indices fit int16.
  - Node pass computes, per OWN node, h = x@W and the attention exponentials
    es=exp(s), es02=exp(0.2 s), ed=exp(d), ed02=exp(0.2 d) via a fused matmul
    x @ [W | Ws | 0.2Ws | Wd | 0.2Wd]; rows packed into 256B bf16 gather
    table rows.  Key identity (exact for leaky_relu slope 0.2):
        exp(leaky_relu(s+d)) = max(es*ed, min(es02*ed02, 1))
    which factorizes src/dst terms and avoids any per-edge transcendentals.
  - The per-core 6250-row table slice (+1 zero pad row) is AllGathered so the
    full 50008-row table lands on every core; src gathers read it directly
    (A half = rows 0:25004, B half = rows 25004:50008; pad rows double as the
    zero rows gathered for empty slots). This keeps per-call kernel inputs
    small (the PJRT arg-staging cost scales with input bytes, and dominated
    the old design that shipped full x to every core).
  - Edge pass per SC: dma_gather rows by src (h,es,es02), dma_gather one row
    per sub-chunk by dst (ed,ed02), compute ex and ex-weighted h rows, then a
    membership matmul (lhsT = one-hot of dst-slot built by is_equal against an
    iota tile) accumulates per-dst sums + denominators in PSUM; result rows are
    dma_scatter_add-ed (CCE add) into a per-layer DRAM accumulator.
  - Edge metadata ships deduplicated (the gather index lists are 16-partition
    wraps replicated 8x on chip, not on the host).
  - Tail applies softmax-denominator, bias and log_softmax.
"""

import math
import os
import sys

import numpy as np

sys.path.insert(0, "/opt/trn_rl_repo")

import ml_dtypes

BF16 = ml_dtypes.bfloat16

# ---------------------------------------------------------------- problem cfg
N_NODES = 50000
N_EDGES = 1600000
IN_CH = 128
C1 = 64  # heads1*dim1
H1 = 8
D1H = 8
C2 = 32
H2 = 1
NEG = 0.2
EPS = 1e-16
NCORES = 8
HALF = 25000  # src-half split so gather idx fits int16
S = 8  # slots per sub-chunk
SCP = 128  # sub-chunks per super-chunk


class Cfg:
    def __init__(self, n_nodes=N_NODES, in_ch=IN_CH, c1=C1, h1=H1, c2=C2,
                 ncores=NCORES, half=HALF):
        self.n_nodes = n_nodes
        self.in_ch = in_ch
        self.c1 = c1
        self.h1 = h1
        self.d1 = c1 // h1
        self.c2 = c2
        self.ncores = ncores
        self.half = half
        self.rows_core = n_nodes // ncores
        self.acc_rows = ((self.rows_core + 127) // 128) * 128 + 256  # + park
        self.park = self.acc_rows - 192
        self.slice_rows = self.rows_core + 1   # own table slice + zero pad row
        self.tab_rows = self.slice_rows * (ncores // 2)  # rows per src half
        self.zrow = self.rows_core  # local index of the pad row in a slice


# ------------------------------------------------------------- host edge plan
def _wrap_idx(idx, reps=16):
    """[n] -> wrapped [16, n/16] layout (pos i at [i%16, i//16])."""
    n = idx.shape[0]
    w = idx.reshape(n // 16, 16).T.copy()  # [16, n/16]
    return np.tile(w, (reps // 16, 1))


def build_plan(src, dst, cfg: Cfg):
    """Per-core, per-stream super-chunk plan. Returns meta [cores, SCN, 16, 80]
    int16 (wrapped idx lists), dsl [cores, SCN, 128] f32, and (scA, scB).

    Gather indices address the AllGathered table: global node g owned by core
    c (= g // rows_core) sits at slice-local row (g - c*rows_core) of slice c,
    i.e. half-local index rel = (g % HALF) + (g % HALF) // rows_core within
    its half; the pad row of the half's first slice (index cfg.zrow) is zero
    and is used for empty slots.
    """
    rows = cfg.rows_core
    half = cfg.half
    core_of = dst // rows
    per_core = []
    for c in range(cfg.ncores):
        m = core_of == c
        s_c = src[m].astype(np.int64)
        d_c = dst[m].astype(np.int64) - c * rows
        stream = (s_c >= half).astype(np.int64)
        order = np.lexsort((s_c, stream, d_c))
        s_c, d_c, stream = s_c[order], d_c[order], stream[order]
        per_core.append((s_c, d_c, stream))

    # sub-chunk lists per (core, stream)
    def subchunks(s_c, d_c, st_c, which):
        m = st_c == which
        s, d = s_c[m], d_c[m]
        if s.shape[0] == 0:
            return (np.zeros((0, S), np.int64), np.zeros((0,), np.int64))
        # group by dst (sorted); ranks within group
        chg = np.r_[True, d[1:] != d[:-1]]
        gid = np.cumsum(chg) - 1
        gstart = np.flatnonzero(chg)
        rank = np.arange(d.shape[0]) - gstart[gid]
        sub_l = rank // S          # sub-chunk index within group
        slot = rank % S
        gsub = np.zeros(gid.max() + 1, np.int64)
        np.maximum.at(gsub, gid, sub_l + 1)          # sub-chunks per group
        gsub_off = np.r_[0, np.cumsum(gsub)]
        subid = gsub_off[gid] + sub_l
        nsub = int(gsub_off[-1])
        src_slots = np.zeros((nsub, S), np.int64)    # pad -> zero row
        src_slots[:] = cfg.zrow
        sh = s - (0 if which == 0 else half)         # half-local node id
        rel = sh + sh // rows                        # skip pad rows
        src_slots[subid, slot] = rel
        sub_dst = np.repeat(d[gstart], gsub)         # dst_rel per sub-chunk
        return src_slots, sub_dst

    plans = []  # per core: list of SC dicts per stream
    maxsc = [0, 0]
    for c in range(cfg.ncores):
        s_c, d_c, st_c = per_core[c]
        streams = []
        for which in (0, 1):
            src_slots, sub_dst = subchunks(s_c, d_c, st_c, which)
            # pack whole dst-groups (consecutive equal sub_dst) into SCs <=128
            scs = []
            n = sub_dst.shape[0]
            i = 0
            cur = []  # list of (start, count) groups
            cur_n = 0
            while i < n:
                j = i
                while j < n and sub_dst[j] == sub_dst[i]:
                    j += 1
                g = j - i
                assert g <= SCP, "dst run too large for one super-chunk"
                if cur_n + g > SCP:
                    scs.append((cur, cur_n))
                    cur, cur_n = [], 0
                cur.append((i, g))
                cur_n += g
                i = j
            if cur_n:
                scs.append((cur, cur_n))
            streams.append((src_slots, sub_dst, scs))
            maxsc[which] = max(maxsc[which], len(scs))
        plans.append(streams)

    scA = ((maxsc[0] + 3) // 4) * 4
    scB = ((maxsc[1] + 3) // 4) * 4
    scn = scA + scB
    meta = np.zeros((cfg.ncores, scn, 16, 80), np.int16)
    dsl = np.full((cfg.ncores, scn, 128), -1.0, np.float32)
    for c in range(cfg.ncores):
        for which in (0, 1):
            src_slots, sub_dst, scs = plans[c][which]
            base = 0 if which == 0 else scA
            nsc = scA if which == 0 else scB
            for k in range(nsc):
                g_idx = np.full((128, S), cfg.zrow, np.int64)
                d_idx = np.full((128,), cfg.park, np.int64)
                dstslot = np.full((128,), -1.0, np.float64)
                scat = np.full((128,), 0, np.int64)
                scat[:] = cfg.park + np.arange(128) % 64
                if k < len(scs):
                    groups, _n = scs[k]
                    p = 0
                    sid = 0
                    for (gs, gc) in groups:
                        g_idx[p:p + gc] = src_slots[gs:gs + gc]
                        d_idx[p:p + gc] = sub_dst[gs]
                        dstslot[p:p + gc] = sid
                        scat[sid] = sub_dst[gs]
                        p += gc
                        sid += 1
                mrow = meta[c, base + k]
                # gather idx list: position i = slot*128 + p
                flat = g_idx.T.reshape(-1)  # [1024] pos-ordered
                mrow[:, 0:64] = _wrap_idx(flat.astype(np.int16))
                mrow[:, 64:72] = _wrap_idx(d_idx.astype(np.int16))
                mrow[:, 72:80] = _wrap_idx(scat.astype(np.int16))
                dsl[c, base + k] = dstslot.astype(np.float32)
    return meta, dsl, scA, scB


# --------------------------------------------------------------- bass program
def build_program(cfg: Cfg, scA, scB, with_cc=True):
    from concourse import bacc, bass, library_config, mybir, tile
    from concourse.masks import make_identity

    f32 = mybir.dt.float32
    bf16 = mybir.dt.bfloat16
    i16 = mybir.dt.int16
    Alu = mybir.AluOpType
    Act = mybir.ActivationFunctionType

    scn = scA + scB
    scg = scn // 4
    nc = bacc.Bacc(None, target_bir_lowering=False, debug=False,
                   num_swdge_queues=4, dynamic_dma_scratch_size=65536)

    # ---- I/O (kept small: per-call PJRT arg staging scales with bytes)
    x_own = nc.dram_tensor("x_own", [cfg.rows_core, cfg.in_ch], bf16, kind="ExternalInput")
    wc1 = nc.dram_tensor("wc1", [cfg.in_ch, 96], f32, kind="ExternalInput")
    wc2 = nc.dram_tensor("wc2", [cfg.c1, 36], f32, kind="ExternalInput")
    b1c = nc.dram_tensor("b1c", [1, cfg.c1], f32, kind="ExternalInput")
    b2c = nc.dram_tensor("b2c", [1, cfg.c2], f32, kind="ExternalInput")
    iota1 = nc.dram_tensor("iota1", [1, 128], f32, kind="ExternalInput")
    meta16_in = nc.dram_tensor("meta16_in", [scg, 16, 320], i16, kind="ExternalInput")
    dsl_in = nc.dram_tensor("dsl_in", [scg, 128, 8], i16, kind="ExternalInput")
    out_ext = nc.dram_tensor("out_ext", [cfg.rows_core, cfg.c2], f32, kind="ExternalOutput")

    # ---- internal DRAM
    ag1in = nc.dram_tensor("ag1in", [cfg.slice_rows, 128], bf16)
    ag2in = nc.dram_tensor("ag2in", [cfg.slice_rows, 128], bf16)
    ago1 = nc.dram_tensor("ago1", [cfg.slice_rows * cfg.ncores, 128], bf16,
                          addr_space="Shared")
    ago2 = nc.dram_tensor("ago2", [cfg.slice_rows * cfg.ncores, 128], bf16,
                          addr_space="Shared")
    d1t = nc.dram_tensor("d1t", [cfg.acc_rows, 128], bf16)
    d2t = nc.dram_tensor("d2t", [cfg.acc_rows, 128], bf16)
    acc1 = nc.dram_tensor("acc1", [cfg.acc_rows, 128], f32)
    acc2 = nc.dram_tensor("acc2", [cfg.acc_rows, 128], f32)

    otile = (cfg.rows_core + 127) // 128

    with tile.TileContext(nc) as tc:
        nc.gpsimd.load_library(library_config.mlp)
        with tc.tile_pool(name="const", bufs=1) as cpool:
            ident = cpool.tile([128, 128], f32)
            make_identity(nc, ident[:])
            wc1_s = cpool.tile([cfg.in_ch, 96], f32)
            nc.sync.dma_start(wc1_s[:], wc1[:, :])
            wc2_s = cpool.tile([cfg.c1, 36], f32)
            nc.sync.dma_start(wc2_s[:], wc2[:, :])
            b1_s = cpool.tile([128, cfg.c1], f32)
            nc.sync.dma_start(b1_s[0:1, :], b1c[:, :])
            nc.gpsimd.partition_broadcast(b1_s[:], b1_s[0:1, :])
            b2_s = cpool.tile([128, cfg.c2], f32)
            nc.sync.dma_start(b2_s[0:1, :], b2c[:, :])
            nc.gpsimd.partition_broadcast(b2_s[:], b2_s[0:1, :])
            iota_s = cpool.tile([128, 128], f32)
            nc.sync.dma_start(iota_s[0:1, :], iota1[:, :])
            nc.gpsimd.partition_broadcast(iota_s[:], iota_s[0:1, :])
            zf = cpool.tile([128, 128], f32)
            nc.vector.memset(zf[:], 0.0)
            zb = cpool.tile([128, 128], bf16)
            nc.vector.memset(zb[:], 0.0)

            # ---- phase 0: zero accumulators + D tables
            # one DMA covers many 128-row tiles via a step-0 repeat of the
            # (fully initialized) zero tile
            def zfill(t, nr, zt):
                o = 0
                while o < nr:
                    full = min(16, (nr - o) // 128)
                    if full:
                        nc.scalar.dma_start(
                            t[o:o + full * 128, :].rearrange(
                                "(a p) c -> p a c", p=128),
                            zt[:].rearrange("p (o c) -> p o c", o=1)
                                .to_broadcast([128, full, 128]))
                        o += full * 128
                    else:
                        nc.scalar.dma_start(t[o:nr, :], zt[:nr - o, :])
                        o = nr
            zfill(acc1, cfg.acc_rows, zf)
            zfill(acc2, cfg.acc_rows, zf)
            for t in (d1t, d2t):
                zfill(t, cfg.acc_rows, zb)
            # pad rows of the AllGather slices (gathered for empty slots)
            nc.scalar.dma_start(ag1in[cfg.zrow:cfg.zrow + 1, :], zb[0:1, :])
            nc.scalar.dma_start(ag2in[cfg.zrow:cfg.zrow + 1, :], zb[0:1, :])

            # ---- phase 1: node pass over OWN rows -> ag1in slice (+ d1 stats)
            def node_pass1(pool, ppool):
                BN = 4

                def compute_tile(r, xt_ap, tb_ap):
                    xts = ppool.tile([cfg.in_ch, 128], f32, tag="xtp")
                    nc.tensor.transpose(out=xts[:, :r], in_=xt_ap[:r, :],
                                        identity=ident[:r, :r])
                    xT = pool.tile([cfg.in_ch, 128], f32, tag="xT")
                    nc.scalar.copy(out=xT[:, :r], in_=xts[:, :r])
                    hp = ppool.tile([128, 96], f32, tag="hp")
                    nc.tensor.matmul(out=hp[:r, :], lhsT=xT[:, :r],
                                     rhs=wc1_s[:], start=True, stop=True)
                    nc.scalar.copy(out=tb_ap[:r, 0:64], in_=hp[:r, 0:64])
                    nc.scalar.activation(out=tb_ap[:r, 64:96],
                                         in_=hp[:r, 64:96], func=Act.Exp)
                    nc.vector.memset(tb_ap[:r, 96:128], 0.0)

                t = 0
                while t < otile:
                    o = t * 128
                    nb = min(BN, otile - t)
                    rows = min(nb * 128, cfg.rows_core - o)
                    if nb == BN and rows == nb * 128:
                        xtb = pool.tile([128, BN, cfg.in_ch], bf16, tag="xtb")
                        nc.sync.dma_start(
                            xtb[:],
                            x_own[o:o + rows, :].rearrange(
                                "(a p) c -> p a c", p=128))
                        xt4 = pool.tile([128, BN, cfg.in_ch], f32, tag="xt4")
                        nc.scalar.copy(out=xt4[:], in_=xtb[:])
                        tb4 = pool.tile([128, BN, 128], bf16, tag="tb4")
                        for j in range(BN):
                            compute_tile(128, xt4[:, j, :], tb4[:, j, :])
                        nc.sync.dma_start(
                            ag1in[o:o + rows, :].rearrange(
                                "(a p) c -> p a c", p=128),
                            tb4[:])
                        nc.sync.dma_start(
                            d1t[o:o + rows, 0:16].rearrange(
                                "(a p) c -> p a c", p=128),
                            tb4[:, :, 80:96])
                        t += BN
                    else:
                        r = min(128, cfg.rows_core - o)
                        xtb = pool.tile([128, cfg.in_ch], bf16, tag="xtb1")
                        nc.sync.dma_start(xtb[:r, :], x_own[o:o + r, :])
                        xt = pool.tile([128, cfg.in_ch], f32, tag="xt1")
                        nc.scalar.copy(out=xt[:r, :], in_=xtb[:r, :])
                        tb = pool.tile([128, 128], bf16, tag="tb1")
                        compute_tile(r, xt[:], tb[:])
                        nc.sync.dma_start(ag1in[o:o + r, :], tb[:r, :])
                        nc.sync.dma_start(d1t[o:o + r, 0:16], tb[:r, 80:96])
                        t += 1

            # ---- edge pass (shared for both layers)
            def edge_pass(tab_a, tab_b, dtab, acc, es_off, nh, hc, pool, ppool):
                dim = hc // nh
                G4 = 4
                scgA = scA // G4
                scgT = scn // G4
                for gi in range(scgT):
                    tab = tab_a if gi < scgA else tab_b
                    mt = pool.tile([128, 328], i16, tag="mt")
                    for r8 in range(8):
                        nc.sync.dma_start(mt[16 * r8:16 * r8 + 16, 0:320],
                                          meta16_in[gi, :, :])
                    nc.sync.dma_start(mt[:, 320:328], dsl_in[gi, :, :])
                    dsl = mt[:, 320:328].bitcast(f32)
                    g = pool.tile([128, G4 * S, 128], bf16, tag="g")
                    for k in range(G4):
                        nc.gpsimd.dma_gather(
                            g[:, k * S:(k + 1) * S, :], tab,
                            mt[:, 64 * k:64 * k + 64],
                            128 * S, 128 * S, 128, queue_num=k)
                    dt_ = pool.tile([128, G4, 128], bf16, tag="dt")
                    nc.gpsimd.dma_gather(
                        dt_[:], dtab[:, :], mt[:, 256:288],
                        G4 * 128, G4 * 128, 128, queue_num=1)
                    gv = g[:].rearrange("p (k s) c -> p k s c", s=S)
                    m = pool.tile([128, G4, 128], bf16, tag="m")
                    for k in range(G4):
                        nc.vector.tensor_scalar(
                            out=m[:, k, :], in0=iota_s[:],
                            scalar1=dsl[:, k:k + 1], scalar2=None,
                            op0=Alu.is_equal)
                    u = pool.tile([128, G4, S, nh], bf16, tag="u")
                    v = pool.tile([128, G4, S, nh], bf16, tag="v")
                    r_ = pool.tile([128, G4 * S, hc + nh], bf16, tag="r")
                    rv = r_[:].rearrange("p (k s) c -> p k s c", s=S)
                    nc.vector.tensor_tensor(
                        out=u[:], in0=gv[:, :, :, es_off:es_off + nh],
                        in1=dt_[:].rearrange("p k (o c) -> p k o c", o=1)
                            [:, :, :, 0:nh].to_broadcast([128, G4, S, nh]),
                        op=Alu.mult)
                    nc.vector.tensor_tensor(
                        out=v[:], in0=gv[:, :, :, es_off + nh:es_off + 2 * nh],
                        in1=dt_[:].rearrange("p k (o c) -> p k o c", o=1)
                            [:, :, :, nh:2 * nh].to_broadcast([128, G4, S, nh]),
                        op=Alu.mult)
                    nc.vector.tensor_scalar(
                        out=v[:], in0=v[:], scalar1=1.0, scalar2=None,
                        op0=Alu.min)
                    nc.vector.tensor_tensor(
                        out=rv[:, :, :, hc:hc + nh], in0=u[:], in1=v[:],
                        op=Alu.max)
                    exb = rv[:, :, :, hc:hc + nh].rearrange(
                        "p k s (h o) -> p (k s) h o", o=1).to_broadcast(
                        [128, G4 * S, nh, dim])
                    nc.vector.tensor_tensor(
                        out=r_[:, :, 0:hc].rearrange(
                            "p c (h d) -> p c h d", d=dim),
                        in0=g[:, :, 0:hc].rearrange(
                            "p c (h d) -> p c h d", d=dim),
                        in1=exb, op=Alu.mult)
                    sc_w = 64 if hc + nh <= 64 else 128
                    sout = pool.tile([128, G4, sc_w], f32, tag="sout")
                    if hc + nh < sc_w:
                        nc.vector.memset(sout[:, :, hc + nh:sc_w], 0.0)
                    for k in range(G4):
                        ps = ppool.tile([128, hc + nh], f32, tag="ps")
                        for s_ in range(S):
                            nc.tensor.matmul(out=ps[:], lhsT=m[:, k, :],
                                             rhs=r_[:, k * S + s_, :],
                                             start=(s_ == 0),
                                             stop=(s_ == S - 1))
                        nc.scalar.copy(out=sout[:, k, 0:hc + nh],
                                       in_=ps[:])
                    nc.gpsimd.dma_scatter_add(
                        acc[:, 0:sc_w], sout[:], mt[:, 288:320],
                        G4 * 128, G4 * 128, sc_w, elem_step=128)

            with tc.tile_pool(name="np1", bufs=6) as np1_pool, \
                 tc.tile_pool(name="np1p", bufs=2, space="PSUM") as np1_ppool:
                node_pass1(np1_pool, np1_ppool)

            # ---- AllGather the layer-1 table slices
            tc.strict_bb_all_engine_barrier()
            if with_cc:
                nc.gpsimd.collective_compute(
                    "AllGather", mybir.AluOpType.bypass,
                    replica_groups=[list(range(cfg.ncores))],
                    ins=[ag1in[:, :]], outs=[ago1[:, :]])
            tc.strict_bb_all_engine_barrier()

            # ---- phase 2: layer-1 edge pass
            with tc.tile_pool(name="ep1", bufs=6) as ep_pool, \
                 tc.tile_pool(name="ep1p", bufs=4, space="PSUM") as ep_ppool:
                edge_pass(ago1[0:cfg.tab_rows, :],
                          ago1[cfg.tab_rows:2 * cfg.tab_rows, :],
                          d1t, acc1, 64, cfg.h1, cfg.c1, ep_pool, ep_ppool)

            # ---- phase 3: layer-2 node pass (local rows)
            # batched path for exactly-4-full-tile groups; leftover per-tile
            with tc.tile_pool(name="np2", bufs=4) as pool, \
                 tc.tile_pool(name="np2p", bufs=4, space="PSUM") as ppool:
                for g in range(otile // 4):
                    o = g * 512
                    ra = pool.tile([128, 4, 72], f32, tag="ra4")
                    nc.sync.dma_start(
                        ra[:], acc1[o:o + 512, 0:72]
                        .rearrange("(a p) c -> p a c", p=128))
                    den = pool.tile([128, 4, cfg.h1], f32, tag="den4")
                    nc.vector.tensor_scalar(out=den[:], in0=ra[:, :, 64:72],
                                            scalar1=EPS, scalar2=None,
                                            op0=Alu.add)
                    rec = pool.tile([128, 4, cfg.h1], f32, tag="rec4")
                    nc.vector.reciprocal(out=rec[:], in_=den[:])
                    h2 = pool.tile([128, 4, cfg.c1], f32, tag="h24")
                    nc.vector.tensor_tensor(
                        out=h2[:].rearrange("p a (h d) -> p a h d", d=cfg.d1),
                        in0=ra[:, :, 0:64].rearrange("p a (h d) -> p a h d",
                                                     d=cfg.d1),
                        in1=rec[:].rearrange("p a (h o) -> p a h o", o=1)
                            .to_broadcast([128, 4, cfg.h1, cfg.d1]),
                        op=Alu.mult)
                    nc.vector.tensor_tensor(
                        out=h2[:], in0=h2[:],
                        in1=b1_s[:, :].rearrange("p (o c) -> p o c", o=1)
                            .to_broadcast([128, 4, cfg.c1]), op=Alu.add)
                    t1_ = pool.tile([128, 4, cfg.c1], f32, tag="t14")
                    nc.vector.tensor_scalar(out=t1_[:], in0=h2[:], scalar1=0.0,
                                            scalar2=None, op0=Alu.min)
                    nc.scalar.activation(out=t1_[:], in_=t1_[:], func=Act.Exp)
                    nc.vector.tensor_scalar(out=h2[:], in0=h2[:], scalar1=0.0,
                                            scalar2=None, op0=Alu.max)
                    nc.vector.tensor_tensor(out=h2[:], in0=h2[:], in1=t1_[:],
                                            op=Alu.add)
                    nc.vector.tensor_scalar(out=h2[:], in0=h2[:], scalar1=-1.0,
                                            scalar2=None, op0=Alu.add)
                    ag = pool.tile([128, 4, 128], bf16, tag="ag4")
                    nc.vector.memset(ag[:, :, 34:128], 0.0)
                    ex4 = pool.tile([128, 4, 4], bf16, tag="ex44")
                    for j in range(4):
                        hts = ppool.tile([cfg.c1, 128], f32, tag="hts")
                        nc.tensor.transpose(out=hts[:], in_=h2[:, j, :],
                                            identity=ident[:])
                        hT = pool.tile([cfg.c1, 128], f32, tag="hT")
                        nc.scalar.copy(out=hT[:], in_=hts[:])
                        p2 = ppool.tile([128, 36], f32, tag="p2")
                        nc.tensor.matmul(out=p2[:], lhsT=hT[:], rhs=wc2_s[:],
                                         start=True, stop=True)
                        nc.vector.tensor_copy(out=ag[:, j, 0:32],
                                              in_=p2[:, 0:32])
                        nc.scalar.activation(out=ex4[:, j, :],
                                             in_=p2[:, 32:36], func=Act.Exp)
                        nc.vector.tensor_copy(out=ag[:, j, 32:34],
                                              in_=ex4[:, j, 0:2])
                    nc.sync.dma_start(
                        ag2in[o:o + 512, :].rearrange("(a p) c -> p a c", p=128),
                        ag[:])
                    nc.sync.dma_start(
                        d2t[o:o + 512, 0:2].rearrange("(a p) c -> p a c", p=128),
                        ex4[:, :, 2:4])
                for t in range(4 * (otile // 4), otile):
                    o = t * 128
                    r = min(128, cfg.rows_core - o)
                    ra = pool.tile([128, 72], f32, tag="ra")
                    nc.sync.dma_start(ra[:r, :], acc1[o:o + r, 0:72])
                    den = pool.tile([128, cfg.h1], f32, tag="den")
                    nc.vector.tensor_scalar(out=den[:r, :], in0=ra[:r, 64:72],
                                            scalar1=EPS, scalar2=None, op0=Alu.add)
                    rec = pool.tile([128, cfg.h1], f32, tag="rec")
                    nc.vector.reciprocal(out=rec[:r, :], in_=den[:r, :])
                    h2 = pool.tile([128, cfg.c1], f32, tag="h2")
                    nc.vector.tensor_tensor(
                        out=h2[:r, :].rearrange("p (h d) -> p h d", d=cfg.d1),
                        in0=ra[:r, 0:64].rearrange("p (h d) -> p h d", d=cfg.d1),
                        in1=rec[:r, :].rearrange("p (h o) -> p h o", o=1)
                            .to_broadcast([r, cfg.h1, cfg.d1]),
                        op=Alu.mult)
                    nc.vector.tensor_tensor(out=h2[:r, :], in0=h2[:r, :],
                                            in1=b1_s[:r, :], op=Alu.add)
                    # ELU: max(x,0) + exp(min(x,0)) - 1
                    t1_ = pool.tile([128, cfg.c1], f32, tag="t1_")
                    nc.vector.tensor_scalar(out=t1_[:r, :], in0=h2[:r, :],
                                            scalar1=0.0, scalar2=None, op0=Alu.min)
                    nc.scalar.activation(out=t1_[:r, :], in_=t1_[:r, :], func=Act.Exp)
                    nc.vector.tensor_scalar(out=h2[:r, :], in0=h2[:r, :],
                                            scalar1=0.0, scalar2=None, op0=Alu.max)
                    nc.vector.tensor_tensor(out=h2[:r, :], in0=h2[:r, :],
                                            in1=t1_[:r, :], op=Alu.add)
                    nc.vector.tensor_scalar(out=h2[:r, :], in0=h2[:r, :],
                                            scalar1=-1.0, scalar2=None, op0=Alu.add)
                    hts = ppool.tile([cfg.c1, 128], f32, tag="hts")
                    nc.tensor.transpose(out=hts[:, :r], in_=h2[:r, :], identity=ident[:r, :r])
                    hT = pool.tile([cfg.c1, 128], f32, tag="hT")
                    nc.scalar.copy(out=hT[:, :r], in_=hts[:, :r])
                    p2 = ppool.tile([128, 36], f32, tag="p2")
                    nc.tensor.matmul(out=p2[:r, :], lhsT=hT[:, :r], rhs=wc2_s[:],
                                     start=True, stop=True)
                    ag = pool.tile([128, 128], bf16, tag="ag")
                    nc.vector.memset(ag[:r, 34:128], 0.0)
                    nc.vector.tensor_copy(out=ag[:r, 0:32], in_=p2[:r, 0:32])
                    ex4 = pool.tile([128, 4], bf16, tag="ex4")
                    nc.scalar.activation(out=ex4[:r, :], in_=p2[:r, 32:36], func=Act.Exp)
                    nc.vector.tensor_copy(out=ag[:r, 32:34], in_=ex4[:r, 0:2])
                    nc.sync.dma_start(ag2in[o:o + r, :], ag[:r, :])
                    nc.sync.dma_start(d2t[o:o + r, 0:2], ex4[:r, 2:4])

            # ---- phase 4: AllGather the layer-2 table slices
            tc.strict_bb_all_engine_barrier()
            if with_cc:
                nc.gpsimd.collective_compute(
                    "AllGather", mybir.AluOpType.bypass,
                    replica_groups=[list(range(cfg.ncores))],
                    ins=[ag2in[:, :]], outs=[ago2[:, :]])
            tc.strict_bb_all_engine_barrier()

            # ---- phase 5: layer-2 edge pass
            with tc.tile_pool(name="ep2", bufs=6) as ep_pool2, \
                 tc.tile_pool(name="ep2p", bufs=6, space="PSUM") as ep_ppool2:
                edge_pass(ago2[0:cfg.tab_rows, :],
                          ago2[cfg.tab_rows:2 * cfg.tab_rows, :],
                          d2t, acc2, 32, H2, cfg.c2, ep_pool2, ep_ppool2)

            # ---- phase 6: tail (normalize + bias + log_softmax), 4 tiles/step
            with tc.tile_pool(name="tl", bufs=4) as pool:
                t = 0
                while t < otile:
                    o = t * 128
                    nb = min(4, otile - t)
                    rows = min(nb * 128, cfg.rows_core - o)
                    if rows < nb * 128:
                        nb -= 1
                        rows = nb * 128
                    if nb >= 1:
                        ra = pool.tile([128, 4, 33], f32, tag="tra")
                        nc.sync.dma_start(
                            ra[:, :nb, :], acc2[o:o + rows, 0:33]
                            .rearrange("(a p) c -> p a c", p=128))
                        den = pool.tile([128, 4], f32, tag="tden")
                        nc.vector.tensor_scalar(
                            out=den[:, :nb], in0=ra[:, :nb, 32], scalar1=EPS,
                            scalar2=None, op0=Alu.add)
                        rec = pool.tile([128, 4], f32, tag="trec")
                        nc.vector.reciprocal(out=rec[:, :nb], in_=den[:, :nb])
                        y = pool.tile([128, 4, 32], f32, tag="ty")
                        nc.vector.tensor_tensor(
                            out=y[:, :nb, :], in0=ra[:, :nb, 0:32],
                            in1=rec[:, :nb].rearrange("p (a o) -> p a o", o=1)
                                .to_broadcast([128, nb, 32]), op=Alu.mult)
                        nc.vector.tensor_tensor(
                            out=y[:, :nb, :], in0=y[:, :nb, :],
                            in1=b2_s[:, :].rearrange("p (o c) -> p o c", o=1)
                                .to_broadcast([128, nb, 32]), op=Alu.add)
                        mx = pool.tile([128, 4], f32, tag="tmx")
                        nc.vector.reduce_max(out=mx[:, :nb], in_=y[:, :nb, :],
                                             axis=mybir.AxisListType.X)
                        nc.vector.tensor_tensor(
                            out=y[:, :nb, :], in0=y[:, :nb, :],
                            in1=mx[:, :nb].rearrange("p (a o) -> p a o", o=1)
                                .to_broadcast([128, nb, 32]), op=Alu.subtract)
                        ey = pool.tile([128, 4, 32], f32, tag="tey")
                        nc.scalar.activation(out=ey[:, :nb, :], in_=y[:, :nb, :],
                                             func=Act.Exp)
                        sm = pool.tile([128, 4], f32, tag="tsm")
                        nc.vector.reduce_sum(out=sm[:, :nb], in_=ey[:, :nb, :],
                                             axis=mybir.AxisListType.X)
                        lg = pool.tile([128, 4], f32, tag="tlg")
                        nc.scalar.activation(out=lg[:, :nb], in_=sm[:, :nb],
                                             func=Act.Ln)
                        nc.vector.tensor_tensor(
                            out=y[:, :nb, :], in0=y[:, :nb, :],
                            in1=lg[:, :nb].rearrange("p (a o) -> p a o", o=1)
                                .to_broadcast([128, nb, 32]), op=Alu.subtract)
                        nc.sync.dma_start(
                            out_ext[o:o + rows, :]
                            .rearrange("(a p) c -> p a c", p=128),
                            y[:, :nb, :])
                        t += nb
                        continue
                    r = cfg.rows_core - o
                    ra1 = pool.tile([128, 33], f32, tag="tra1")
                    nc.sync.dma_start(ra1[:r, :], acc2[o:o + r, 0:33])
                    den1 = pool.tile([128, 1], f32, tag="tden1")
                    nc.vector.tensor_scalar(out=den1[:r, :], in0=ra1[:r, 32:33],
                                            scalar1=EPS, scalar2=None, op0=Alu.add)
                    rec1 = pool.tile([128, 1], f32, tag="trec1")
                    nc.vector.reciprocal(out=rec1[:r, :], in_=den1[:r, :])
                    y1 = pool.tile([128, cfg.c2], f32, tag="ty1")
                    nc.vector.tensor_scalar(out=y1[:r, :], in0=ra1[:r, 0:32],
                                            scalar1=rec1[:r, :], scalar2=None,
                                            op0=Alu.mult)
                    nc.vector.tensor_tensor(out=y1[:r, :], in0=y1[:r, :],
                                            in1=b2_s[:r, :], op=Alu.add)
                    mx1 = pool.tile([128, 1], f32, tag="tmx1")
                    nc.vector.reduce_max(out=mx1[:r, :], in_=y1[:r, :],
                                         axis=mybir.AxisListType.X)
                    nc.vector.tensor_scalar(out=y1[:r, :], in0=y1[:r, :],
                                            scalar1=mx1[:r, :], scalar2=None,
                                            op0=Alu.subtract)
                    ey1 = pool.tile([128, cfg.c2], f32, tag="tey1")
                    nc.scalar.activation(out=ey1[:r, :], in_=y1[:r, :], func=Act.Exp)
                    sm1 = pool.tile([128, 1], f32, tag="tsm1")
                    nc.vector.reduce_sum(out=sm1[:r, :], in_=ey1[:r, :],
                                         axis=mybir.AxisListType.X)
                    lg1 = pool.tile([128, 1], f32, tag="tlg1")
                    nc.scalar.activation(out=lg1[:r, :], in_=sm1[:r, :], func=Act.Ln)
                    nc.vector.tensor_scalar(out=y1[:r, :], in0=y1[:r, :],
                                            scalar1=lg1[:r, :], scalar2=None,
                                            op0=Alu.subtract)
                    nc.sync.dma_start(out_ext[o:o + r, :], y1[:r, :])
                    t += 1

    if not nc.is_finalized():
        nc.finalize()
    return nc


# -------------------------------------------------------------------- weights
def host_weights(W1, a_src1, a_dst1, W2, a_src2, a_dst2, cfg: Cfg):
    # Ws[k,h] = sum_d W1[k, h*dim+d] * a_src1[h, d]
    W1r = W1.reshape(cfg.in_ch, cfg.h1, cfg.d1)
    Ws1 = np.einsum("khd,hd->kh", W1r, a_src1)
    Wd1 = np.einsum("khd,hd->kh", W1r, a_dst1)
    wc1 = np.concatenate([W1, Ws1, NEG * Ws1, Wd1, NEG * Wd1], axis=1).astype(np.float32)
    Ws2 = (W2 @ a_src2[0][:, None])  # [c1,1]
    Wd2 = (W2 @ a_dst2[0][:, None])
    wc2 = np.concatenate([W2, Ws2, NEG * Ws2, Wd2, NEG * Wd2], axis=1).astype(np.float32)
    return wc1, wc2


def host_inputs(x, edge_index, W1, a_src1, a_dst1, b1, W2, a_src2, a_dst2, b2,
                cfg: Cfg):
    n = cfg.n_nodes
    loops = np.arange(n, dtype=np.int64)
    src = np.concatenate([np.asarray(edge_index[0], np.int64), loops])
    dst = np.concatenate([np.asarray(edge_index[1], np.int64), loops])
    meta, dsl, scA, scB = build_plan(src, dst, cfg)
    # repack into 4-SC groups:
    #   meta16 [scg,16,320] = gidx [16,256] | didx [16,32] | scat [16,32]
    #   dsl16  [scg,128,8]  = dstslot f32 x4 viewed as int16
    scg = (scA + scB) // 4
    meta16 = np.zeros((cfg.ncores, scg, 16, 320), np.int16)
    dsl16 = np.zeros((cfg.ncores, scg, 128, 8), np.int16)
    for g in range(scg):
        for k in range(4):
            m = meta[:, 4 * g + k]
            meta16[:, g, :, 64 * k:64 * k + 64] = m[:, :, 0:64]
            meta16[:, g, :, 256 + 8 * k:256 + 8 * k + 8] = m[:, :, 64:72]
            meta16[:, g, :, 288 + 8 * k:288 + 8 * k + 8] = m[:, :, 72:80]
            dsl16[:, g, :, 2 * k:2 * k + 2] = (
                dsl[:, 4 * g + k].astype(np.float32).view(np.int16)
                .reshape(cfg.ncores, 128, 2))
    wc1, wc2 = host_weights(np.asarray(W1, np.float32), np.asarray(a_src1, np.float32),
                            np.asarray(a_dst1, np.float32), np.asarray(W2, np.float32),
                            np.asarray(a_src2, np.float32), np.asarray(a_dst2, np.float32),
                            cfg)
    iota = np.arange(128, dtype=np.float32)[None, :]
    x = np.asarray(x, np.float32).astype(BF16)
    in_maps = []
    for c in range(cfg.ncores):
        in_maps.append({
            "x_own": np.ascontiguousarray(x[c * cfg.rows_core:(c + 1) * cfg.rows_core]),
            "wc1": wc1, "wc2": wc2,
            "b1c": np.asarray(b1, np.float32)[None, :],
            "b2c": np.asarray(b2, np.float32)[None, :],
            "iota1": np.ascontiguousarray(iota),
            "meta16_in": np.ascontiguousarray(meta16[c]),
            "dsl_in": np.ascontiguousarray(dsl16[c]),
        })
    return in_maps, scA, scB


_CACHE = {}
LAST_RESULT = None


def kernel(**inputs) -> np.ndarray:
    cfg = Cfg()
    in_maps, scA, scB = host_inputs(cfg=cfg, **inputs)
    key = (scA, scB)
    if key not in _CACHE:
        _CACHE[key] = build_program(cfg, scA, scB)
    nc = _CACHE[key]
    from concourse.bass_utils import run_bass_kernel_spmd
    trace = bool(int(os.environ.get("GAT_TRACE", "0")))
    res = run_bass_kernel_spmd(nc, in_maps, list(range(cfg.ncores)), trace=trace)
    global LAST_RESULT
    LAST_RESULT = res
    outs = [np.asarray(res.results[c]["out_ext"], np.float32)
            for c in range(cfg.ncores)]
    return np.concatenate(outs, axis=0)


if __name__ == "__main__":
    pass
